# revision 1
# baseline (speedup 1.0000x reference)
"""XNOR-Net BasicBlock forward (BN-sign-binconv-PReLU x2 + BN + residual + PReLU)
distributed over 8 Trainium2 NeuronCores, data-parallel over the batch axis.

Self-contained: hardcodes shapes N=64, C=128, H=W=56, 8 cores.
"""

import numpy as np
import ml_dtypes

import concourse.bass as bass
import concourse.mybir as mybir
import concourse.tile as tile
from concourse import bacc
from concourse.bass_utils import run_bass_kernel_spmd

F32 = mybir.dt.float32
F16 = mybir.dt.float16
BF16 = mybir.dt.bfloat16
FP8 = mybir.dt.float8e4
PITCH = 64
AF = mybir.ActivationFunctionType
OP = mybir.AluOpType

N_CORES = 8
N_LOC = 8          # images per core
C = 128            # channels (== partitions)
H = W = 56
HW = H * W         # 3136
PADW = W + 2       # 58
EPS = 1e-5
TILE_ROWS = 7      # output rows per PSUM tile -> 7*64-span = 448 <= 512 (one bank)
N_TILES = H // TILE_ROWS   # 8 -> 4 uniform groups of 2, 4-deep PSUM rotation
CHUNK = TILE_ROWS * W      # 392
N_CHUNKS = HW // CHUNK     # 8

# pp param columns
P_S1, P_S2, P_G1, P_B1, P_G2, P_B2, P_G3, P_B3, P_A1, P_A2, P_A3 = range(11)
NP = 11


def _col(t, j):
    return t[:, j : j + 1]


def _rstd_from_allreduced(nc, pool, ar, name):
    """ar: [128,2] = sum over cores of [mean_i, var_i + mean_i^2].
    Returns (mean, rstd) tiles [128,1] f32 with rstd = 1/sqrt(var+EPS),
    Newton-refined to cover ScalarE Sqrt spline error."""
    mean = pool.tile([C, 1], F32, name=f"mean_{name}", tag=f"mean_{name}")
    ex2 = pool.tile([C, 1], F32, name=f"ex2_{name}", tag="sc_ex2")
    nc.vector.tensor_scalar_mul(mean[:], _col(ar, 0), 1.0 / N_CORES)
    nc.vector.tensor_scalar_mul(ex2[:], _col(ar, 1), 1.0 / N_CORES)
    negmean = pool.tile([C, 1], F32, name=f"negmean_{name}", tag="sc_negmean")
    nc.vector.tensor_scalar_mul(negmean[:], mean[:], -1.0)
    vpe = pool.tile([C, 1], F32, name=f"vpe_{name}", tag="sc_vpe")
    # vpe = ex2 - mean^2 + EPS  == (mean * -mean) add ex2, then +EPS
    nc.vector.scalar_tensor_tensor(vpe[:], mean[:], negmean[:], ex2[:], OP.mult, OP.add)
    nc.vector.tensor_scalar_add(vpe[:], vpe[:], EPS)
    rec = pool.tile([C, 1], F32, name=f"rec_{name}", tag="sc_rec")
    nc.vector.reciprocal(rec[:], vpe[:])
    rstd = pool.tile([C, 1], F32, name=f"rstd_{name}", tag=f"rstd_{name}")
    nc.scalar.activation(rstd[:], rec[:], AF.Sqrt)
    # Newton: y <- y * (1.5 - 0.5 * vpe * y^2)
    t1 = pool.tile([C, 1], F32, name=f"t1_{name}", tag="sc_t1")
    nc.vector.tensor_tensor(out=t1[:], in0=rstd[:], in1=rstd[:], op=OP.mult)
    nc.vector.tensor_tensor(out=t1[:], in0=t1[:], in1=vpe[:], op=OP.mult)
    nc.vector.tensor_scalar(t1[:], t1[:], -0.5, 1.5, OP.mult, OP.add)
    nc.vector.tensor_tensor(out=rstd[:], in0=rstd[:], in1=t1[:], op=OP.mult)
    return mean, rstd


def _affine_consts(nc, pool, pp, mean, rstd, g_col, b_col, name):
    """k = g * rstd ; cb = b - mean * k. Returns (k, cb) tiles [128,1]."""
    k = pool.tile([C, 1], F32, name=f"k_{name}", tag=f"k_{name}")
    nc.vector.tensor_tensor(out=k[:], in0=_col(pp, g_col), in1=rstd[:], op=OP.mult)
    negk = pool.tile([C, 1], F32, name=f"negk_{name}", tag="sc_negk")
    nc.vector.tensor_scalar_mul(negk[:], k[:], -1.0)
    cb = pool.tile([C, 1], F32, name=f"cb_{name}", tag=f"cb_{name}")
    nc.vector.scalar_tensor_tensor(
        cb[:], mean[:], negk[:], _col(pp, b_col), OP.mult, OP.add
    )
    return k, cb


def _sign_threshold(nc, pool, k, cb, ra, rs, name):
    """b = sign(k*prelu(s*c) + cb) == Sign(c*sgn - sgn*tau) for monotone prelu
    (a>0). ra=1/a, rs=1/s precomputed. Returns (sgn, nbias) [128,1] tiles."""
    negcb = pool.tile([C, 1], F32, name=f"negcb_{name}", tag="sc_negcb")
    nc.vector.tensor_scalar_mul(negcb[:], cb[:], -1.0)
    rk = pool.tile([C, 1], F32, name=f"rk_{name}", tag="sc_rk")
    nc.vector.reciprocal(rk[:], k[:])
    t2 = pool.tile([C, 1], F32, name=f"t2_{name}", tag="sc_t2")
    nc.vector.tensor_tensor(out=t2[:], in0=negcb[:], in1=rk[:], op=OP.mult)
    # prelu^-1(t2) = max(t2,0) + min(t2,0)/a
    tpos = pool.tile([C, 1], F32, name=f"tpos_{name}", tag="sc_tpos")
    nc.vector.tensor_scalar_max(tpos[:], t2[:], 0.0)
    tneg = pool.tile([C, 1], F32, name=f"tneg_{name}", tag="sc_tneg")
    nc.vector.tensor_scalar_min(tneg[:], t2[:], 0.0)
    pinv = pool.tile([C, 1], F32, name=f"pinv_{name}", tag="sc_pinv")
    nc.vector.scalar_tensor_tensor(pinv[:], tneg[:], ra[:], tpos[:],
                                   OP.mult, OP.add)
    tau = pool.tile([C, 1], F32, name=f"tau_{name}", tag="sc_tau")
    nc.vector.tensor_tensor(out=tau[:], in0=pinv[:], in1=rs[:], op=OP.mult)
    sgn = pool.tile([C, 1], F32, name=f"sgn_{name}", tag=f"sgn_{name}")
    nc.scalar.activation(sgn[:], k[:], AF.Sign)
    nbias = pool.tile([C, 1], F32, name=f"nbias_{name}", tag=f"nbias_{name}")
    nc.vector.tensor_tensor(out=nbias[:], in0=sgn[:], in1=tau[:], op=OP.mult)
    nc.vector.tensor_scalar_mul(nbias[:], nbias[:], -1.0)
    return sgn, nbias


def build_nc(dbg=False, reps=1):
    nc = bacc.Bacc(None, target_bir_lowering=False, debug=False, num_devices=N_CORES)

    x_d = nc.dram_tensor("x", [N_LOC, C, HW], F32, kind="ExternalInput")
    xb_d = nc.dram_tensor("xb", [N_LOC, C, HW], F16, kind="ExternalInput")
    w1_d = nc.dram_tensor("w1t", [9, C, C], FP8, kind="ExternalInput")
    w2_d = nc.dram_tensor("w2t", [9, C, C], FP8, kind="ExternalInput")
    pp_d = nc.dram_tensor("pp", [C, NP], F32, kind="ExternalInput")
    out_d = nc.dram_tensor("out", [N_LOC, C, HW], F16, kind="ExternalOutput")
    if dbg:
        dbg_pad_d = nc.dram_tensor("dbg_pad", [C, H + 2, PITCH], FP8,
                                   kind="ExternalOutput")
        dbg_c1_d = nc.dram_tensor("dbg_c1", [C, N_LOC, HW], F16,
                                  kind="ExternalOutput")
        dbg_c2_d = nc.dram_tensor("dbg_c2", [C, N_LOC, HW], F16,
                                  kind="ExternalOutput")
        dbg_k_d = nc.dram_tensor("dbg_k", [C, 10], F32, kind="ExternalOutput")

    with tile.TileContext(nc) as tc:
        with (
            tc.tile_pool(name="const", bufs=1) as const,
            tc.tile_pool(name="work", bufs=2) as work,
            tc.tile_pool(name="psum", bufs=2, space="PSUM") as psum,
            tc.tile_pool(name="dram", bufs=1, space="DRAM") as dram,
        ):
            # ---- persistent SBUF tensors ----
            pp = const.tile([C, NP], F32)
            nc.gpsimd.dma_start(pp[:], pp_d[:])
            w1s = const.tile([C, 9, C], FP8)
            w2s = const.tile([C, 9, C], FP8)
            for t in range(9):
                nc.gpsimd.dma_start(w1s[:, t, :], w1_d[t])
                nc.gpsimd.dma_start(w2s[:, t, :], w2_d[t])
            c1f = const.tile([C, N_LOC, HW], F16)   # conv1 raw integer outputs
            c2f = const.tile([C, N_LOC, HW], F16)   # conv2 raw integer outputs
            stats1 = const.tile([C, N_LOC * N_CHUNKS, 6], F32, tag="stats")
            stats2 = const.tile([C, N_LOC * N_CHUNKS, 6], F32, tag="stats")
            stats3 = const.tile([C, N_LOC * N_CHUNKS, 6], F32, tag="stats")
            pads = []
            for j in range(2):
                # +1 spare zero row: tile-6 dh=2 taps read 2 elements past
                # row 57 for garbage output columns (skipped at evacuation)
                p = const.tile([C, H + 3, PITCH], FP8, name=f"pad{j}")
                nc.vector.memset(p[:], 0.0)
                pads.append(p)

            a1 = _col(pp, P_A1)
            a2 = _col(pp, P_A2)
            a3 = _col(pp, P_A3)
            s1 = _col(pp, P_S1)
            s2 = _col(pp, P_S2)

            ra1 = const.tile([C, 1], F32, name="ra1")
            nc.vector.reciprocal(ra1[:], a1)
            rs1 = const.tile([C, 1], F32, name="rs1")
            nc.vector.reciprocal(rs1[:], s1)

            cc_counter = [0]

            def reduce_stats(stats, idx):
                """bn_aggr + pack [mean, var+mean^2] + allreduce; returns [128,2] tile."""
                mv = const.tile([C, 2], F32, name=f"mv{idx}", tag="sc_mv")
                nc.vector.bn_aggr(mv[:], stats[:])
                e = const.tile([C, 2], F32, name=f"e{idx}", tag="sc_e")
                nc.vector.tensor_copy(_col(e, 0), _col(mv, 0))
                nc.vector.scalar_tensor_tensor(
                    _col(e, 1), _col(mv, 0), _col(mv, 0), _col(mv, 1), OP.mult, OP.add
                )
                n = cc_counter[0]
                cc_counter[0] += 1
                cci = dram.tile([C, 2], F32, name=f"cc_in{n}", tag=f"cc_in{n}")
                cco = dram.tile([N_CORES, C, 2], F32, name=f"cc_out{n}",
                                tag=f"cc_out{n}", addr_space="Shared")
                nc.sync.dma_start(cci[:], e[:])
                nc.gpsimd.collective_compute(
                    "AllGather",
                    OP.bypass,
                    replica_groups=[list(range(N_CORES))],
                    ins=[cci.opt()],
                    outs=[cco.opt()],
                )
                g8 = const.tile([C, 2, N_CORES], F32, name=f"g8{idx}", tag="sc_g8")
                for r in range(N_CORES):
                    nc.sync.dma_start(g8[:, :, r], cco[r])
                g = const.tile([C, 2], F32, name=f"g{idx}", tag="sc_g")
                nc.vector.tensor_reduce(g[:], g8[:], mybir.AxisListType.X, OP.add)
                return g

            QSPAN = TILE_ROWS * PITCH  # 512: flat padded span per tile

            def conv(pad, ws, dst, stats_to=None):
                """3x3 conv of padded +/-1 fp8 image (row pitch 64) with 9 [C,C]
                taps -> dst [C,HW] f16. Vertical tap pairs (dh=0,1) run as fp8
                DoubleRow matmuls (256-deep contraction); dh=2 taps run as
                plain fp8 matmuls. Outputs computed over the flat padded span
                (8 garbage cols per row skipped at evacuation)."""
                padf = pad[:].rearrange("p r w -> p (r w)")
                wbase = ws[:, 0, :]
                for g0 in range(0, N_TILES, 2):
                    tiles = range(g0, min(g0 + 2, N_TILES))
                    ng = len(tiles)
                    # one PSUM tile spanning the group's banks (512 f32 = 1 bank each)
                    # each sub-tile padded to a full 512-elem bank so the
                    # matmul target never crosses a bank boundary
                    psg = psum.tile([C, ng, 512], F32,
                                    tag="ps",
                                    name=f"psg{g0 // 2}", bufs=3)
                    for dw in range(3):
                        # pair lhsT: taps (0,dw) and (1,dw); tap stride = 3*C
                        wp = bass.AP(wbase.tensor, wbase.offset + dw * C,
                                     [list(wbase.ap[0]), [3 * C, 2], [1, C]])
                        for j, t in enumerate(tiles):
                            q0 = t * QSPAN + dw
                            rhs = bass.AP(padf.tensor, padf.offset + q0,
                                          [list(padf.ap[0]), [PITCH, 2],
                                           [1, QSPAN]])
                            nc.tensor.matmul(
                                psg[:, j, 0:QSPAN], wp, rhs, start=(dw == 0),
                                stop=False,
                                perf_mode=mybir.MatmulPerfMode.DoubleRow,
                            )
                    for dw in range(3):
                        for j, t in enumerate(tiles):
                            q0 = t * QSPAN + 2 * PITCH + dw
                            nc.tensor.matmul(
                                psg[:, j, 0:QSPAN], ws[:, 6 + dw, :],
                                padf[:, q0 : q0 + QSPAN],
                                start=False, stop=(dw == 2),
                            )
                    # single strided evacuation for the whole group;
                    # first group of each image goes via ScalarE (DVE relief)
                    gbase = psg[:]
                    src_ap = bass.AP(gbase.tensor, gbase.offset,
                                     [list(gbase.ap[0]), [512, ng],
                                      [PITCH, TILE_ROWS], [1, W]])
                    dst_ap = dst[:, g0 * CHUNK : (g0 + ng) * CHUNK].rearrange(
                        "p (g r w) -> p g r w", r=TILE_ROWS, w=W)
                    nc.vector.tensor_copy(dst_ap, src_ap)
                    if stats_to is not None:
                        stats, i, sc, al = stats_to
                        pst = work.tile([C, 2 * CHUNK], F32, tag="pstat",
                                        bufs=2)
                        nc.scalar.activation(
                            pst[:, 0 : ng * CHUNK],
                            dst[:, g0 * CHUNK : (g0 + ng) * CHUNK],
                            AF.Prelu, scale=sc, alpha=al)
                        for j, t in enumerate(tiles):
                            nc.vector.bn_stats(
                                stats[:, i * N_CHUNKS + t, :],
                                pst[:, j * CHUNK : (j + 1) * CHUNK])

            def image_stats(src, stats, i):
                for cch in range(N_CHUNKS):
                    nc.vector.bn_stats(
                        stats[:, i * N_CHUNKS + cch, :],
                        src[:, cch * CHUNK : (cch + 1) * CHUNK],
                    )

            for _rep in range(reps):
                # ================= Phase A: BN1 stats =================
                for i in range(N_LOC):
                    xin = work.tile([C, HW], F32, tag="xin", bufs=3)
                    q = HW // 4
                    for qq in range(4):
                        nc.sync.dma_start(xin[:, qq * q : (qq + 1) * q],
                                          x_d[i, :, qq * q : (qq + 1) * q])
                    image_stats(xin, stats1, i)

                g1ar = reduce_stats(stats1, 0)
                mean1, rstd1 = _rstd_from_allreduced(nc, const, g1ar, "1")
                k1, c1b = _affine_consts(nc, const, pp, mean1, rstd1, P_G1, P_B1, "1")

                # ================= Phase B: b1 = sign(BN1(x)); conv1; stats2 ========
                for i in range(N_LOC):
                    xin = work.tile([C, HW], F32, tag="xin", bufs=3)
                    qb = HW // 4
                    for qq in range(4):
                        nc.sync.dma_start(xin[:, qq * qb : (qq + 1) * qb],
                                          x_d[i, :, qq * qb : (qq + 1) * qb])
                    pad = pads[i % 2]
                    nc.scalar.activation(
                        pad[:, 1 : H + 1, 1 : W + 1],
                        xin.rearrange("p (h w) -> p h w", h=H, w=W),
                        AF.Sign,
                        bias=c1b[:],
                        scale=k1[:],
                    )
                    if dbg and i == 0:
                        nc.sync.dma_start(dbg_pad_d[:], pad[:])
                    conv(pad, w1s, c1f[:, i, :], stats_to=(stats2, i, s1[:], a1[:]))

                g2ar = reduce_stats(stats2, 1)
                mean2, rstd2 = _rstd_from_allreduced(nc, const, g2ar, "2")
                k2, c2b = _affine_consts(nc, const, pp, mean2, rstd2, P_G2, P_B2, "2")

                # ================= Phase C: b2 = sign(BN2(p1)); conv2; stats3 =======
                sgn2, nbias2 = _sign_threshold(nc, const, k2, c2b, ra1[:], rs1[:], f"2")
                for i in range(N_LOC):
                    pad = pads[i % 2]
                    nc.scalar.activation(
                        pad[:, 1 : H + 1, 1 : W + 1],
                        c1f[:, i, :].rearrange("p (h w) -> p h w", h=H, w=W),
                        AF.Sign,
                        bias=nbias2[:],
                        scale=sgn2[:],
                    )
                    conv(pad, w2s, c2f[:, i, :], stats_to=(stats3, i, s2[:], a2[:]))

                g3ar = reduce_stats(stats3, 2)
                mean3, rstd3 = _rstd_from_allreduced(nc, const, g3ar, "3")
                k3, c3b = _affine_consts(nc, const, pp, mean3, rstd3, P_G3, P_B3, "3")

                if dbg:
                    nc.sync.dma_start(dbg_c1_d[:], c1f[:])
                    nc.sync.dma_start(dbg_c2_d[:], c2f[:])
                    dbgk = const.tile([C, 10], F32)
                    for j, t_ in enumerate(
                        [k1, c1b, k2, c2b, k3, c3b, mean1, rstd1, mean2, rstd2]
                    ):
                        nc.vector.tensor_copy(_col(dbgk, j), t_[:])
                    nc.sync.dma_start(dbg_k_d[:], dbgk[:])

                # ====== Phase D: y = PReLU(BN3(PReLU(s2*c2)) + x) ======
                for i in range(N_LOC):
                    xbt = work.tile([C, HW], F16, tag="xbt", bufs=2)
                    nc.sync.dma_start(xbt[:], xb_d[i])
                    p2t = work.tile([C, HW], F16, tag="f32a", bufs=2)
                    nc.scalar.activation(
                        p2t[:], c2f[:, i, :], AF.Prelu, scale=s2[:], alpha=a2[:]
                    )
                    wv = work.tile([C, HW], F32, tag="f32b", bufs=2)
                    nc.vector.scalar_tensor_tensor(
                        wv[:], p2t[:], k3[:], xbt[:], OP.mult, OP.add
                    )
                    yout = work.tile([C, HW], F16, tag="xin", bufs=3)
                    nc.scalar.activation(
                        yout[:], wv[:], AF.Prelu, bias=c3b[:], alpha=a3[:]
                    )
                    nc.sync.dma_start(out_d[i], yout[:])

    nc.compile()
    return nc


def _prep_host(x, bn1_g, bn1_b, w1, prelu1_a, bn2_g, bn2_b, w2, prelu2_a,
               bn3_g, bn3_b, prelu3_a):
    def wprep(w_flat):
        w = np.asarray(w_flat, np.float32).reshape(C, C, 3, 3)
        scale = np.mean(np.abs(w), axis=(1, 2, 3)).astype(np.float32)  # [C]
        # lhsT layout [tap, i, o] = sign(w[o, i, dh, dw])
        wT = np.sign(w).transpose(2, 3, 1, 0).reshape(9, C, C)
        return wT.astype(mybir.dt.np(FP8)), scale

    w1t, s1 = wprep(w1)
    w2t, s2 = wprep(w2)

    pp = np.zeros((C, NP), np.float32)
    pp[:, P_S1] = s1
    pp[:, P_S2] = s2
    pp[:, P_G1] = np.asarray(bn1_g, np.float32)
    pp[:, P_B1] = np.asarray(bn1_b, np.float32)
    pp[:, P_G2] = np.asarray(bn2_g, np.float32)
    pp[:, P_B2] = np.asarray(bn2_b, np.float32)
    pp[:, P_G3] = np.asarray(bn3_g, np.float32)
    pp[:, P_B3] = np.asarray(bn3_b, np.float32)
    pp[:, P_A1] = np.float32(prelu1_a)
    pp[:, P_A2] = np.float32(prelu2_a)
    pp[:, P_A3] = np.float32(prelu3_a)

    x = np.ascontiguousarray(np.asarray(x, np.float32).reshape(64, C, HW))
    xb = x.astype(np.float16)
    in_maps = []
    for r in range(N_CORES):
        in_maps.append({
            "x": x[r * N_LOC : (r + 1) * N_LOC],
            "xb": xb[r * N_LOC : (r + 1) * N_LOC],
            "w1t": w1t,
            "w2t": w2t,
            "pp": pp,
        })
    return in_maps


_NC_CACHE = None


def _get_nc():
    global _NC_CACHE
    if _NC_CACHE is None:
        _NC_CACHE = build_nc()
    return _NC_CACHE


def run(in_maps, **kwargs):
    nc = _get_nc()
    return run_bass_kernel_spmd(nc, in_maps, core_ids=list(range(N_CORES)), **kwargs)


def kernel(**inputs):
    in_maps = _prep_host(**inputs)
    last_err = None
    for attempt in range(3):
        try:
            res = run(in_maps)
            break
        except Exception as e:  # transient NRT device errors happen; retry
            last_err = e
            import time as _time
            _time.sleep(2.0)
    else:
        raise last_err
    out = np.concatenate(
        [np.asarray(r["out"]).astype(np.float32).reshape(N_LOC, C, H, W)
         for r in res.results], axis=0
    )
    return out


if __name__ == "__main__":
    rng = np.random.default_rng(0)
    x = rng.standard_normal((64, C, H, W), dtype=np.float32)
    w1 = ((rng.random((C * C * 9, 1), dtype=np.float32) - 0.5) * 0.002)
    w2 = ((rng.random((C * C * 9, 1), dtype=np.float32) - 0.5) * 0.002)
    ones = np.ones(C, np.float32)
    zeros = np.zeros(C, np.float32)
    y = kernel(x=x, bn1_g=ones, bn1_b=zeros, w1=w1, prelu1_a=np.float32(0.25),
               bn2_g=ones, bn2_b=zeros, w2=w2, prelu2_a=np.float32(0.25),
               bn3_g=ones, bn3_b=zeros, prelu3_a=np.float32(0.25))
    print("out", y.shape, y.dtype, float(np.abs(y).mean()))



# revision 36
# speedup vs baseline: 1.2234x; 1.2234x over previous
"""XNOR-Net BasicBlock forward (BN-sign-binconv-PReLU x2 + BN + residual + PReLU)
distributed over 8 Trainium2 NeuronCores, data-parallel over the batch axis.

Self-contained: hardcodes shapes N=64, C=128, H=W=56, 8 cores.
"""

import numpy as np
import ml_dtypes

import concourse.bass as bass
import concourse.mybir as mybir
import concourse.tile as tile
from concourse import bacc
from concourse.bass_utils import run_bass_kernel_spmd

F32 = mybir.dt.float32
F16 = mybir.dt.float16
BF16 = mybir.dt.bfloat16
FP8 = mybir.dt.float8e4
PITCH = 64
AF = mybir.ActivationFunctionType
OP = mybir.AluOpType

N_CORES = 8
N_LOC = 8          # images per core
C = 128            # channels (== partitions)
H = W = 56
HW = H * W         # 3136
EPS = 1e-5
TILE_ROWS = 7      # output rows per PSUM bank span: 7*64 = 448 <= 512
N_TILES = H // TILE_ROWS   # 8 tiles -> 2 groups of 4 banks
QSPAN = TILE_ROWS * PITCH  # 448
CHUNK = TILE_ROWS * W      # 392
HALF = HW // 2             # 1568
# padded image: 59 rows x 64 pitch = 3776; second copy at +PADX where
# (PADX + 1) % 16 == 0 so the dh=2 horizontal tap pair is DoubleRow-legal
PADX = 3791
PAIR_DH2 = True

# pp param columns
P_S1, P_S2, P_G1, P_B1, P_G2, P_B2, P_G3, P_B3, P_A1, P_A2, P_A3 = range(11)
NP = 11


def _col(t, j):
    return t[:, j : j + 1]


def _rstd_from_allreduced(nc, pool, ar, name):
    """ar: [128,2] = sum over cores of [mean_i, var_i + mean_i^2].
    Returns (mean, rstd) tiles [128,1] f32 with rstd = 1/sqrt(var+EPS),
    Newton-refined to cover ScalarE Sqrt spline error."""
    mean = pool.tile([C, 1], F32, name=f"mean_{name}", tag=f"mean_{name}")
    ex2 = pool.tile([C, 1], F32, name=f"ex2_{name}", tag="sc_ex2")
    nc.vector.tensor_scalar_mul(mean[:], _col(ar, 0), 1.0 / N_CORES)
    nc.vector.tensor_scalar_mul(ex2[:], _col(ar, 1), 1.0 / N_CORES)
    negmean = pool.tile([C, 1], F32, name=f"negmean_{name}", tag="sc_negmean")
    nc.vector.tensor_scalar_mul(negmean[:], mean[:], -1.0)
    vpe = pool.tile([C, 1], F32, name=f"vpe_{name}", tag="sc_vpe")
    nc.vector.scalar_tensor_tensor(vpe[:], mean[:], negmean[:], ex2[:], OP.mult, OP.add)
    nc.vector.tensor_scalar_add(vpe[:], vpe[:], EPS)
    rec = pool.tile([C, 1], F32, name=f"rec_{name}", tag="sc_rec")
    nc.vector.reciprocal(rec[:], vpe[:])
    rstd = pool.tile([C, 1], F32, name=f"rstd_{name}", tag=f"rstd_{name}")
    nc.scalar.activation(rstd[:], rec[:], AF.Sqrt)
    # Newton: y <- y * (1.5 - 0.5 * vpe * y^2)
    t1 = pool.tile([C, 1], F32, name=f"t1_{name}", tag="sc_t1")
    nc.vector.tensor_tensor(out=t1[:], in0=rstd[:], in1=rstd[:], op=OP.mult)
    nc.vector.tensor_tensor(out=t1[:], in0=t1[:], in1=vpe[:], op=OP.mult)
    nc.vector.tensor_scalar(t1[:], t1[:], -0.5, 1.5, OP.mult, OP.add)
    nc.vector.tensor_tensor(out=rstd[:], in0=rstd[:], in1=t1[:], op=OP.mult)
    return mean, rstd


def _affine_consts(nc, pool, pp, mean, rstd, g_col, b_col, name):
    """k = g * rstd ; cb = b - mean * k. Returns (k, cb) tiles [128,1]."""
    k = pool.tile([C, 1], F32, name=f"k_{name}", tag=f"k_{name}")
    nc.vector.tensor_tensor(out=k[:], in0=_col(pp, g_col), in1=rstd[:], op=OP.mult)
    negk = pool.tile([C, 1], F32, name=f"negk_{name}", tag="sc_negk")
    nc.vector.tensor_scalar_mul(negk[:], k[:], -1.0)
    cb = pool.tile([C, 1], F32, name=f"cb_{name}", tag=f"cb_{name}")
    nc.vector.scalar_tensor_tensor(
        cb[:], mean[:], negk[:], _col(pp, b_col), OP.mult, OP.add
    )
    return k, cb


def _sign_threshold(nc, pool, k, cb, ra, rs, name):
    """b = sign(k*prelu(s*c) + cb) == Sign(c*sgn - sgn*tau) for monotone prelu
    (a>0). ra=1/a, rs=1/s precomputed. Returns (sgn, nbias) [128,1] tiles."""
    negcb = pool.tile([C, 1], F32, name=f"negcb_{name}", tag="sc_negcb")
    nc.vector.tensor_scalar_mul(negcb[:], cb[:], -1.0)
    rk = pool.tile([C, 1], F32, name=f"rk_{name}", tag="sc_rk")
    nc.vector.reciprocal(rk[:], k[:])
    t2 = pool.tile([C, 1], F32, name=f"t2_{name}", tag="sc_t2")
    nc.vector.tensor_tensor(out=t2[:], in0=negcb[:], in1=rk[:], op=OP.mult)
    # prelu^-1(t2) = max(t2,0) + min(t2,0)/a
    tpos = pool.tile([C, 1], F32, name=f"tpos_{name}", tag="sc_tpos")
    nc.vector.tensor_scalar_max(tpos[:], t2[:], 0.0)
    tneg = pool.tile([C, 1], F32, name=f"tneg_{name}", tag="sc_tneg")
    nc.vector.tensor_scalar_min(tneg[:], t2[:], 0.0)
    pinv = pool.tile([C, 1], F32, name=f"pinv_{name}", tag="sc_pinv")
    nc.vector.scalar_tensor_tensor(pinv[:], tneg[:], ra[:], tpos[:],
                                   OP.mult, OP.add)
    tau = pool.tile([C, 1], F32, name=f"tau_{name}", tag="sc_tau")
    nc.vector.tensor_tensor(out=tau[:], in0=pinv[:], in1=rs[:], op=OP.mult)
    sgn = pool.tile([C, 1], F32, name=f"sgn_{name}", tag=f"sgn_{name}")
    nc.scalar.activation(sgn[:], k[:], AF.Sign)
    nbias = pool.tile([C, 1], F32, name=f"nbias_{name}", tag=f"nbias_{name}")
    nc.vector.tensor_tensor(out=nbias[:], in0=sgn[:], in1=tau[:], op=OP.mult)
    nc.vector.tensor_scalar_mul(nbias[:], nbias[:], -1.0)
    return sgn, nbias


def build_nc(reps=1):
    nc = bacc.Bacc(None, target_bir_lowering=False, debug=False, num_devices=N_CORES)

    x_d = nc.dram_tensor("x", [N_LOC, C, HW], F32, kind="ExternalInput")
    w1_d = nc.dram_tensor("w1t", [9, C, C], FP8, kind="ExternalInput")
    w2_d = nc.dram_tensor("w2t", [9, C, C], FP8, kind="ExternalInput")
    pp_d = nc.dram_tensor("pp", [C, NP], F32, kind="ExternalInput")
    out_d = nc.dram_tensor("out", [N_LOC, C, HW], F16, kind="ExternalOutput")

    with tile.TileContext(nc) as tc:
        with (
            tc.tile_pool(name="const", bufs=1) as const,
            tc.tile_pool(name="work", bufs=2) as work,
            tc.tile_pool(name="psum", bufs=2, space="PSUM") as psum,
            tc.tile_pool(name="dram", bufs=1, space="DRAM") as dram,
        ):
            # ---- persistent SBUF tensors ----
            pp = const.tile([C, NP], F32)
            nc.gpsimd.dma_start(pp[:], pp_d[:])
            w1s = const.tile([C, 9, C], FP8)
            w2s = const.tile([C, 9, C], FP8)
            for ws, wd in ((w1s, w1_d), (w2s, w2_d)):
                wv = wd[:]
                src = bass.AP(wv.tensor, wv.offset,
                              [[C, C], [C * C, 9], [1, C]])
                nc.gpsimd.dma_start(ws[:], src)
            xf16 = const.tile([C, N_LOC, HW], F16)   # residual copy of x
            # one shared buffer: phase B writes q1 = prelu(conv1) (EXACT in
            # f16: conv1 is even ints <=1152, a=0.25 a power of two); phase C
            # overwrites image i with p2 = prelu(s2*conv2) after sign2(i)
            # consumed it (ACT program order guarantees the WAR ordering)
            cf = const.tile([C, N_LOC, HW], F16)
            stats1 = const.tile([C, N_LOC * 8, 6], F32, tag="st1")
            stats2 = const.tile([C, N_LOC * 8, 6], F32, tag="st2")
            stats3 = const.tile([C, N_LOC * 8, 6], F32, tag="st3")
            # pads memset-ed after phase A (runs during collective-1 idle)
            pads = [const.tile([C, 2, PADX], FP8, name=f"pad{j}")
                    for j in range(2)]

            a1 = _col(pp, P_A1)
            a2 = _col(pp, P_A2)
            a3 = _col(pp, P_A3)
            s1 = _col(pp, P_S1)
            s2 = _col(pp, P_S2)

            s1sq = const.tile([C, 1], F32, name="s1sq")
            nc.vector.tensor_tensor(out=s1sq[:], in0=s1, in1=s1, op=OP.mult)

            cc_counter = [0]

            def reduce_stats(stats, idx, s_col=None, s2_col=None):
                """bn_aggr + pack [mean, var+mean^2] (optionally rescaled
                from q- to p-domain by s / s^2) + allgather + local reduce;
                returns [128,2] tile of cross-core sums."""
                mv = const.tile([C, 2], F32, name=f"mv{idx}", tag="sc_mv")
                nc.vector.bn_aggr(mv[:], stats[:])
                e = const.tile([C, 2], F32, name=f"e{idx}", tag="sc_e")
                nc.vector.scalar_tensor_tensor(
                    _col(e, 1), _col(mv, 0), _col(mv, 0), _col(mv, 1), OP.mult, OP.add
                )
                if s_col is not None:
                    nc.vector.tensor_tensor(out=_col(e, 0), in0=_col(mv, 0),
                                            in1=s_col, op=OP.mult)
                    nc.vector.tensor_tensor(out=_col(e, 1), in0=_col(e, 1),
                                            in1=s2_col, op=OP.mult)
                else:
                    nc.vector.tensor_copy(_col(e, 0), _col(mv, 0))
                n = cc_counter[0]
                cc_counter[0] += 1
                cci = dram.tile([C, 2], F32, name=f"cc_in{n}", tag=f"cc_in{n}")
                cco = dram.tile([N_CORES, C, 2], F32, name=f"cc_out{n}",
                                tag=f"cc_out{n}", addr_space="Shared")
                nc.sync.dma_start(cci[:], e[:])
                nc.gpsimd.collective_compute(
                    "AllGather",
                    OP.bypass,
                    replica_groups=[list(range(N_CORES))],
                    ins=[cci.opt()],
                    outs=[cco.opt()],
                )
                g8 = const.tile([C, 2, N_CORES], F32, name=f"g8{idx}", tag="sc_g8")
                cav = cco[:]  # AP over [8, C, 2] dram tensor
                src = bass.AP(cav.tensor, cav.offset,
                              [[2, C], [1, 2], [2 * C, N_CORES]])
                nc.sync.dma_start(g8[:], src)
                g = const.tile([C, 2], F32, name=f"g{idx}", tag="sc_g")
                nc.vector.tensor_reduce(g[:], g8[:], mybir.AxisListType.X, OP.add)
                return g

            def conv(pad, ws, i, stats, s_col, a_col):
                """3x3 conv of padded +/-1 fp8 image (pitch 64, duplicate copy
                at +PADX for the dh=2 pair) with 9 [C,C] taps; 2 groups of 4
                PSUM banks. Evacuation = single ACT Prelu from PSUM -> cf
                (scale s_col folded in when given); bn_stats read cf."""
                ng = 4
                padf = pad[:].rearrange("p two x -> p (two x)")
                wbase = ws[:, 0, :]
                for g in range(N_TILES // ng):
                    psg = psum.tile([C, ng, 512], F32, tag="ps",
                                    name=f"psg{g}", bufs=8 // ng)
                    for dw in range(3):
                        wp = bass.AP(wbase.tensor, wbase.offset + dw * C,
                                     [list(wbase.ap[0]), [3 * C, 2], [1, C]])
                        for t in range(ng):
                            q0 = (g * ng + t) * QSPAN + dw
                            rhs = bass.AP(padf.tensor, padf.offset + q0,
                                          [list(padf.ap[0]), [PITCH, 2],
                                           [1, QSPAN]])
                            nc.tensor.matmul(
                                psg[:, t, 0:QSPAN], wp, rhs, start=(dw == 0),
                                stop=False,
                                perf_mode=mybir.MatmulPerfMode.DoubleRow,
                            )
                    if PAIR_DH2:
                        # taps (2,0)+(2,1) as one DR pair: second element read
                        # from the duplicate pad copy at +PADX+1 (mult of 16)
                        wp2 = bass.AP(wbase.tensor, wbase.offset + 6 * C,
                                      [list(wbase.ap[0]), [C, 2], [1, C]])
                        for t in range(ng):
                            q0 = (g * ng + t) * QSPAN + 2 * PITCH
                            rhs = bass.AP(padf.tensor, padf.offset + q0,
                                          [list(padf.ap[0]), [PADX + 1, 2],
                                           [1, QSPAN]])
                            nc.tensor.matmul(
                                psg[:, t, 0:QSPAN], wp2, rhs, start=False,
                                stop=False,
                                perf_mode=mybir.MatmulPerfMode.DoubleRow,
                            )
                        last_dws = (2,)
                    else:
                        last_dws = (0, 1, 2)
                    for li, dw in enumerate(last_dws):
                        for t in range(ng):
                            q0 = (g * ng + t) * QSPAN + 2 * PITCH + dw
                            nc.tensor.matmul(
                                psg[:, t, 0:QSPAN], ws[:, 6 + dw, :],
                                padf[:, q0 : q0 + QSPAN],
                                start=False, stop=(li == len(last_dws) - 1),
                            )
                    gbase = psg[:]

                    def sub_ap(t0, nt):
                        src = bass.AP(gbase.tensor, gbase.offset + t0 * 512,
                                      [list(gbase.ap[0]), [512, nt],
                                       [PITCH, TILE_ROWS], [1, W]])
                        d = cf[:, i, (g * ng + t0) * CHUNK
                               : (g * ng + t0 + nt) * CHUNK].rearrange(
                            "p (t r w) -> p t r w", r=TILE_ROWS, w=W)
                        return src, d

                    fine = (i == N_LOC - 1 and g == 1)
                    pieces = ((0, 2), (2, 2)) if fine else ((0, ng),)
                    for p0, nt in pieces:
                        src, d = sub_ap(p0, nt)
                        kw = {} if s_col is None else {"scale": s_col}
                        nc.scalar.activation(d, src, AF.Prelu,
                                             alpha=a_col, **kw)
                        for t in range(nt):
                            cch = i * 8 + g * ng + p0 + t
                            nc.vector.bn_stats(
                                stats[:, cch, :],
                                cf[:, i, (g * ng + p0 + t) * CHUNK
                                   : (g * ng + p0 + t + 1) * CHUNK])

            def sign_to_pad(pad, src_img, scale, bias):
                """ACT Sign(scale*src + bias) -> pad interior; then duplicate
                the padded span into the second copy (Pool engine, no DMA)."""
                pv = pad[:, 0, 0 : 59 * PITCH].rearrange(
                    "p (h w) -> p h w", h=59, w=PITCH)
                nc.scalar.activation(
                    pv[:, 1 : H + 1, 1 : W + 1], src_img,
                    AF.Sign, bias=bias, scale=scale)
                if PAIR_DH2:
                    nc.gpsimd.tensor_copy(pad[:, 1, 0 : 59 * PITCH],
                                          pad[:, 0, 0 : 59 * PITCH])

            for _rep in range(reps):
                # ============ Phase A: load x, BN1 stats, f16 residual ======
                QTR = HW // 4  # 784 = 2*CHUNK
                for i in range(N_LOC):
                    xin = work.tile([C, HW], F32, tag="x32", bufs=4)
                    if i == N_LOC - 1:
                        # finer DMA pieces: the tail stats start sooner
                        for qq in range(4):
                            nc.sync.dma_start(
                                xin[:, qq * QTR : (qq + 1) * QTR],
                                x_d[i, :, qq * QTR : (qq + 1) * QTR])
                    else:
                        nc.sync.dma_start(xin[:], x_d[i])
                    for q in range(8):
                        nc.vector.bn_stats(
                            stats1[:, i * 8 + q, :],
                            xin[:, q * CHUNK : (q + 1) * CHUNK])
                    for qq in range(4):
                        cvt = (nc.vector if qq == 3 else nc.gpsimd)
                        cvt.tensor_copy(
                            xf16[:, i, qq * QTR : (qq + 1) * QTR],
                            xin[:, qq * QTR : (qq + 1) * QTR])

                if _rep == 0:
                    # zero only the border cells of half 0: interior is
                    # rewritten per image, half 1 is refreshed by the copies
                    for p in pads:
                        nc.vector.memset(p[:, 0, 0:PITCH], 0.0)  # top row
                        nc.vector.memset(
                            p[:, 0, 57 * PITCH : 59 * PITCH], 0.0)  # bottom
                        pv = p[:, 0, 0 : 59 * PITCH]
                        col = bass.AP(pv.tensor, pv.offset + PITCH,
                                      [list(pv.ap[0]), [PITCH, 56], [1, 1]])
                        nc.vector.memset(col, 0.0)  # left pad col
                        colr = bass.AP(pv.tensor, pv.offset + PITCH + W + 1,
                                       [list(pv.ap[0]), [PITCH, 56], [1, 7]])
                        nc.vector.memset(colr, 0.0)  # right pad cols

                g1ar = reduce_stats(stats1, 0)
                mean1, rstd1 = _rstd_from_allreduced(nc, const, g1ar, "1")
                k1, c1b = _affine_consts(nc, const, pp, mean1, rstd1, P_G1, P_B1, "1")

                # ============ Phase B: sign(BN1(x)); conv1 -> q1; stats2 ====
                # sign/copy for image i+1 are issued before conv(i) so the
                # ACT queue never stalls PE on the next image's pad
                def sign1(i):
                    pad = pads[i % 2]
                    xin = work.tile([C, HW], F32, tag="x32", bufs=4)
                    nc.sync.dma_start(xin[:], x_d[i])
                    pv = pad[:, 0, 0 : 59 * PITCH].rearrange(
                        "p (h w) -> p h w", h=59, w=PITCH)
                    # split at row 30: the lo pad copy only needs rows <31
                    for r0, r1 in ((0, 30), (30, 56)):
                        nc.scalar.activation(
                            pv[:, 1 + r0 : 1 + r1, 1 : W + 1],
                            xin[:, r0 * W : r1 * W].rearrange(
                                "p (h w) -> p h w", h=r1 - r0, w=W),
                            AF.Sign, bias=c1b[:], scale=k1[:])
                        if PAIR_DH2:
                            lo, hi = (0, 31 * PITCH) if r0 == 0 else \
                                     (31 * PITCH, 59 * PITCH)
                            nc.gpsimd.tensor_copy(pad[:, 1, lo:hi],
                                                  pad[:, 0, lo:hi])

                sign1(0)
                for i in range(N_LOC):
                    if i + 1 < N_LOC:
                        sign1(i + 1)
                    conv(pads[i % 2], w1s, i, stats2, None, a1)

                g2ar = reduce_stats(stats2, 1, s1, s1sq[:])
                mean2, rstd2 = _rstd_from_allreduced(nc, const, g2ar, "2")
                k2, c2b = _affine_consts(nc, const, pp, mean2, rstd2, P_G2, P_B2, "2")
                # sign2 = Sign(k2*p1 + cb2) = Sign((k2*s1)*q1 + cb2)
                ks2 = const.tile([C, 1], F32, name="ks2")
                nc.vector.tensor_tensor(out=ks2[:], in0=k2[:], in1=s1, op=OP.mult)

                # ============ Phase C: sign2(q1); conv2 -> p2; stats3 =======
                def sign2(i):
                    pad = pads[i % 2]
                    pv = pad[:, 0, 0 : 59 * PITCH].rearrange(
                        "p (h w) -> p h w", h=59, w=PITCH)
                    for r0, r1 in ((0, 30), (30, 56)):
                        nc.scalar.activation(
                            pv[:, 1 + r0 : 1 + r1, 1 : W + 1],
                            cf[:, i, r0 * W : r1 * W].rearrange(
                                "p (h w) -> p h w", h=r1 - r0, w=W),
                            AF.Sign, bias=c2b[:], scale=ks2[:])
                        if PAIR_DH2:
                            lo, hi = (0, 31 * PITCH) if r0 == 0 else \
                                     (31 * PITCH, 59 * PITCH)
                            nc.gpsimd.tensor_copy(pad[:, 1, lo:hi],
                                                  pad[:, 0, lo:hi])

                sign2(0)
                for i in range(N_LOC):
                    if i + 1 < N_LOC:
                        sign2(i + 1)
                    conv(pads[i % 2], w2s, i, stats3, s2, a2)

                g3ar = reduce_stats(stats3, 2)
                mean3, rstd3 = _rstd_from_allreduced(nc, const, g3ar, "3")
                k3, c3b = _affine_consts(nc, const, pp, mean3, rstd3, P_G3, P_B3, "3")

                # ====== Phase D: y = PReLU(k3*p2 + x + c3b) =================
                # z = k3*p2 + x in ONE stt; c3b folds into the prelu bias
                for i in range(N_LOC):
                    for hh in range(2):
                        sl = slice(hh * HALF, (hh + 1) * HALF)
                        z = work.tile([C, HALF], F16, tag="z16", bufs=4)
                        nc.vector.scalar_tensor_tensor(
                            z[:], cf[:, i, sl], k3[:], xf16[:, i, sl],
                            OP.mult, OP.add)
                        y = work.tile([C, HALF], F16, tag="y16", bufs=4)
                        nc.scalar.activation(y[:], z[:], AF.Prelu,
                                             bias=c3b[:], alpha=a3[:])
                        nc.sync.dma_start(out_d[i, :, sl], y[:])

    nc.compile()
    return nc


def _prep_host(x, bn1_g, bn1_b, w1, prelu1_a, bn2_g, bn2_b, w2, prelu2_a,
               bn3_g, bn3_b, prelu3_a):
    def wprep(w_flat):
        w = np.asarray(w_flat, np.float32).reshape(C, C, 3, 3)
        scale = np.mean(np.abs(w), axis=(1, 2, 3)).astype(np.float32)  # [C]
        # lhsT layout [tap, i, o] = sign(w[o, i, dh, dw])
        wT = np.sign(w).transpose(2, 3, 1, 0).reshape(9, C, C)
        return wT.astype(mybir.dt.np(FP8)), scale

    w1t, s1 = wprep(w1)
    w2t, s2 = wprep(w2)

    pp = np.zeros((C, NP), np.float32)
    pp[:, P_S1] = s1
    pp[:, P_S2] = s2
    pp[:, P_G1] = np.asarray(bn1_g, np.float32)
    pp[:, P_B1] = np.asarray(bn1_b, np.float32)
    pp[:, P_G2] = np.asarray(bn2_g, np.float32)
    pp[:, P_B2] = np.asarray(bn2_b, np.float32)
    pp[:, P_G3] = np.asarray(bn3_g, np.float32)
    pp[:, P_B3] = np.asarray(bn3_b, np.float32)
    pp[:, P_A1] = np.float32(prelu1_a)
    pp[:, P_A2] = np.float32(prelu2_a)
    pp[:, P_A3] = np.float32(prelu3_a)

    x = np.ascontiguousarray(np.asarray(x, np.float32).reshape(64, C, HW))
    in_maps = []
    for r in range(N_CORES):
        in_maps.append({
            "x": x[r * N_LOC : (r + 1) * N_LOC],
            "w1t": w1t,
            "w2t": w2t,
            "pp": pp,
        })
    return in_maps


_NC_CACHE = None


def _get_nc():
    global _NC_CACHE
    if _NC_CACHE is None:
        _NC_CACHE = build_nc()
    return _NC_CACHE


def run(in_maps, **kwargs):
    nc = _get_nc()
    return run_bass_kernel_spmd(nc, in_maps, core_ids=list(range(N_CORES)), **kwargs)


def kernel(**inputs):
    in_maps = _prep_host(**inputs)
    last_err = None
    for attempt in range(3):
        try:
            res = run(in_maps)
            break
        except Exception as e:  # transient NRT device errors happen; retry
            last_err = e
            import time as _time
            _time.sleep(2.0)
    else:
        raise last_err
    out = np.concatenate(
        [np.asarray(r["out"]).astype(np.float32).reshape(N_LOC, C, H, W)
         for r in res.results], axis=0
    )
    return out


if __name__ == "__main__":
    rng = np.random.default_rng(0)
    x = rng.standard_normal((64, C, H, W), dtype=np.float32)
    w1 = ((rng.random((C * C * 9, 1), dtype=np.float32) - 0.5) * 0.002)
    w2 = ((rng.random((C * C * 9, 1), dtype=np.float32) - 0.5) * 0.002)
    ones = np.ones(C, np.float32)
    zeros = np.zeros(C, np.float32)
    y = kernel(x=x, bn1_g=ones, bn1_b=zeros, w1=w1, prelu1_a=np.float32(0.25),
               bn2_g=ones, bn2_b=zeros, w2=w2, prelu2_a=np.float32(0.25),
               bn3_g=ones, bn3_b=zeros, prelu3_a=np.float32(0.25))
    print("out", y.shape, y.dtype, float(np.abs(y).mean()))


# revision 43
# speedup vs baseline: 1.2581x; 1.0283x over previous
"""XNOR-Net BasicBlock forward (BN-sign-binconv-PReLU x2 + BN + residual + PReLU)
distributed over 8 Trainium2 NeuronCores, data-parallel over the batch axis.

Self-contained: hardcodes shapes N=64, C=128, H=W=56, 8 cores.
"""

import numpy as np
import ml_dtypes

import concourse.bass as bass
import concourse.mybir as mybir
import concourse.tile as tile
from concourse import bacc
from concourse.bass_utils import run_bass_kernel_spmd

F32 = mybir.dt.float32
F16 = mybir.dt.float16
BF16 = mybir.dt.bfloat16
FP8 = mybir.dt.float8e4
PITCH = 64
AF = mybir.ActivationFunctionType
OP = mybir.AluOpType

N_CORES = 8
N_LOC = 8          # images per core
C = 128            # channels (== partitions)
H = W = 56
HW = H * W         # 3136
EPS = 1e-5
TILE_ROWS = 7      # output rows per PSUM bank span: 7*64 = 448 <= 512
N_TILES = H // TILE_ROWS   # 8 tiles -> 2 groups of 4 banks
QSPAN = TILE_ROWS * PITCH  # 448
CHUNK = TILE_ROWS * W      # 392
HALF = HW // 2             # 1568
# padded image: 59 rows x 64 pitch = 3776; second copy at +PADX where
# (PADX + 1) % 16 == 0 so the dh=2 horizontal tap pair is DoubleRow-legal
PADX = 3791
PAIR_DH2 = True

# pp param columns
P_S1, P_S2, P_G1, P_B1, P_G2, P_B2, P_G3, P_B3, P_A1, P_A2, P_A3 = range(11)
NP = 11


def _col(t, j):
    return t[:, j : j + 1]


def _rstd_from_allreduced(nc, pool, ar, name):
    """ar: [128,2] = sum over cores of [mean_i, var_i + mean_i^2].
    Returns (mean, rstd) tiles [128,1] f32 with rstd = 1/sqrt(var+EPS),
    Newton-refined to cover ScalarE Sqrt spline error."""
    mean = pool.tile([C, 1], F32, name=f"mean_{name}", tag=f"mean_{name}")
    ex2 = pool.tile([C, 1], F32, name=f"ex2_{name}", tag="sc_ex2")
    nc.vector.tensor_scalar_mul(mean[:], _col(ar, 0), 1.0 / N_CORES)
    nc.vector.tensor_scalar_mul(ex2[:], _col(ar, 1), 1.0 / N_CORES)
    negmean = pool.tile([C, 1], F32, name=f"negmean_{name}", tag="sc_negmean")
    nc.vector.tensor_scalar_mul(negmean[:], mean[:], -1.0)
    vpe = pool.tile([C, 1], F32, name=f"vpe_{name}", tag="sc_vpe")
    nc.vector.scalar_tensor_tensor(vpe[:], mean[:], negmean[:], ex2[:], OP.mult, OP.add)
    nc.vector.tensor_scalar_add(vpe[:], vpe[:], EPS)
    rec = pool.tile([C, 1], F32, name=f"rec_{name}", tag="sc_rec")
    nc.vector.reciprocal(rec[:], vpe[:])
    rstd = pool.tile([C, 1], F32, name=f"rstd_{name}", tag=f"rstd_{name}")
    nc.scalar.activation(rstd[:], rec[:], AF.Sqrt)
    # Newton: y <- y * (1.5 - 0.5 * vpe * y^2)
    t1 = pool.tile([C, 1], F32, name=f"t1_{name}", tag="sc_t1")
    nc.vector.tensor_tensor(out=t1[:], in0=rstd[:], in1=rstd[:], op=OP.mult)
    nc.vector.tensor_tensor(out=t1[:], in0=t1[:], in1=vpe[:], op=OP.mult)
    nc.vector.tensor_scalar(t1[:], t1[:], -0.5, 1.5, OP.mult, OP.add)
    nc.vector.tensor_tensor(out=rstd[:], in0=rstd[:], in1=t1[:], op=OP.mult)
    return mean, rstd


def _affine_consts(nc, pool, pp, mean, rstd, g_col, b_col, name):
    """k = g * rstd ; cb = b - mean * k. Returns (k, cb) tiles [128,1]."""
    k = pool.tile([C, 1], F32, name=f"k_{name}", tag=f"k_{name}")
    nc.vector.tensor_tensor(out=k[:], in0=_col(pp, g_col), in1=rstd[:], op=OP.mult)
    negk = pool.tile([C, 1], F32, name=f"negk_{name}", tag="sc_negk")
    nc.vector.tensor_scalar_mul(negk[:], k[:], -1.0)
    cb = pool.tile([C, 1], F32, name=f"cb_{name}", tag=f"cb_{name}")
    nc.vector.scalar_tensor_tensor(
        cb[:], mean[:], negk[:], _col(pp, b_col), OP.mult, OP.add
    )
    return k, cb


def _sign_threshold(nc, pool, k, cb, ra, rs, name):
    """b = sign(k*prelu(s*c) + cb) == Sign(c*sgn - sgn*tau) for monotone prelu
    (a>0). ra=1/a, rs=1/s precomputed. Returns (sgn, nbias) [128,1] tiles."""
    negcb = pool.tile([C, 1], F32, name=f"negcb_{name}", tag="sc_negcb")
    nc.vector.tensor_scalar_mul(negcb[:], cb[:], -1.0)
    rk = pool.tile([C, 1], F32, name=f"rk_{name}", tag="sc_rk")
    nc.vector.reciprocal(rk[:], k[:])
    t2 = pool.tile([C, 1], F32, name=f"t2_{name}", tag="sc_t2")
    nc.vector.tensor_tensor(out=t2[:], in0=negcb[:], in1=rk[:], op=OP.mult)
    # prelu^-1(t2) = max(t2,0) + min(t2,0)/a
    tpos = pool.tile([C, 1], F32, name=f"tpos_{name}", tag="sc_tpos")
    nc.vector.tensor_scalar_max(tpos[:], t2[:], 0.0)
    tneg = pool.tile([C, 1], F32, name=f"tneg_{name}", tag="sc_tneg")
    nc.vector.tensor_scalar_min(tneg[:], t2[:], 0.0)
    pinv = pool.tile([C, 1], F32, name=f"pinv_{name}", tag="sc_pinv")
    nc.vector.scalar_tensor_tensor(pinv[:], tneg[:], ra[:], tpos[:],
                                   OP.mult, OP.add)
    tau = pool.tile([C, 1], F32, name=f"tau_{name}", tag="sc_tau")
    nc.vector.tensor_tensor(out=tau[:], in0=pinv[:], in1=rs[:], op=OP.mult)
    sgn = pool.tile([C, 1], F32, name=f"sgn_{name}", tag=f"sgn_{name}")
    nc.scalar.activation(sgn[:], k[:], AF.Sign)
    nbias = pool.tile([C, 1], F32, name=f"nbias_{name}", tag=f"nbias_{name}")
    nc.vector.tensor_tensor(out=nbias[:], in0=sgn[:], in1=tau[:], op=OP.mult)
    nc.vector.tensor_scalar_mul(nbias[:], nbias[:], -1.0)
    return sgn, nbias


def build_nc(reps=1):
    nc = bacc.Bacc(None, target_bir_lowering=False, debug=False, num_devices=N_CORES)

    x_d = nc.dram_tensor("x", [N_LOC, C, HW], F32, kind="ExternalInput")
    w1_d = nc.dram_tensor("w1t", [9, C, C], FP8, kind="ExternalInput")
    w2_d = nc.dram_tensor("w2t", [9, C, C], FP8, kind="ExternalInput")
    pp_d = nc.dram_tensor("pp", [C, NP], F32, kind="ExternalInput")
    out_d = nc.dram_tensor("out", [N_LOC, C, HW], F16, kind="ExternalOutput")

    with tile.TileContext(nc) as tc:
        with (
            tc.tile_pool(name="const", bufs=1) as const,
            tc.tile_pool(name="work", bufs=2) as work,
            tc.tile_pool(name="psum", bufs=2, space="PSUM") as psum,
            tc.tile_pool(name="dram", bufs=1, space="DRAM") as dram,
        ):
            # ---- persistent SBUF tensors (loads issued after the first
            # x image so phase A's DMA stream starts immediately) ----
            pp = const.tile([C, NP], F32)
            w1s = const.tile([C, 9, C], FP8)
            w2s = const.tile([C, 9, C], FP8)

            def load_params():
                nc.gpsimd.dma_start(pp[:], pp_d[:])
                for ws, wd in ((w1s, w1_d), (w2s, w2_d)):
                    wv = wd[:]
                    srcw = bass.AP(wv.tensor, wv.offset,
                                   [[C, C], [C * C, 9], [1, C]])
                    nc.gpsimd.dma_start(ws[:], srcw)
            xf16 = const.tile([C, N_LOC, HW], F16)   # residual copy of x
            # one shared buffer: phase B writes q1 = prelu(conv1) (EXACT in
            # f16: conv1 is even ints <=1152, a=0.25 a power of two); phase C
            # overwrites image i with p2 = prelu(s2*conv2) after sign2(i)
            # consumed it (ACT program order guarantees the WAR ordering)
            cf = const.tile([C, N_LOC, HW], F16)
            stats1 = const.tile([C, N_LOC * 8, 6], F32, tag="st1")
            stats2 = const.tile([C, N_LOC * 8, 6], F32, tag="st2")
            stats3 = const.tile([C, N_LOC * 8, 6], F32, tag="st3")
            # pads memset-ed after phase A (runs during collective-1 idle)
            pads = [const.tile([C, 2, PADX], FP8, name=f"pad{j}")
                    for j in range(2)]

            a1 = _col(pp, P_A1)
            a2 = _col(pp, P_A2)
            a3 = _col(pp, P_A3)
            s1 = _col(pp, P_S1)
            s2 = _col(pp, P_S2)

            s1sq = const.tile([C, 1], F32, name="s1sq")

            cc_counter = [0]

            def reduce_stats(stats, idx, s_col=None, s2_col=None):
                """bn_aggr + pack [mean, var+mean^2] (optionally rescaled
                from q- to p-domain by s / s^2) + allgather + local reduce;
                returns [128,2] tile of cross-core sums."""
                mv = const.tile([C, 2], F32, name=f"mv{idx}", tag="sc_mv")
                nc.vector.bn_aggr(mv[:], stats[:])
                e = const.tile([C, 2], F32, name=f"e{idx}", tag="sc_e")
                nc.vector.scalar_tensor_tensor(
                    _col(e, 1), _col(mv, 0), _col(mv, 0), _col(mv, 1), OP.mult, OP.add
                )
                if s_col is not None:
                    nc.vector.tensor_tensor(out=_col(e, 0), in0=_col(mv, 0),
                                            in1=s_col, op=OP.mult)
                    nc.vector.tensor_tensor(out=_col(e, 1), in0=_col(e, 1),
                                            in1=s2_col, op=OP.mult)
                else:
                    nc.vector.tensor_copy(_col(e, 0), _col(mv, 0))
                n = cc_counter[0]
                cc_counter[0] += 1
                cci = dram.tile([C, 2], F32, name=f"cc_in{n}", tag=f"cc_in{n}")
                cco = dram.tile([N_CORES, C, 2], F32, name=f"cc_out{n}",
                                tag=f"cc_out{n}", addr_space="Shared")
                nc.sync.dma_start(cci[:], e[:])
                nc.gpsimd.collective_compute(
                    "AllGather",
                    OP.bypass,
                    replica_groups=[list(range(N_CORES))],
                    ins=[cci.opt()],
                    outs=[cco.opt()],
                )
                g8 = const.tile([C, N_CORES, 2], F32, name=f"g8{idx}", tag="sc_g8")
                cav = cco[:]  # AP over [8, C, 2] dram tensor
                src = bass.AP(cav.tensor, cav.offset,
                              [[2, C], [2 * C, N_CORES], [1, 2]])
                nc.sync.dma_start(g8[:], src)  # contiguous 8B runs
                g = const.tile([C, 2], F32, name=f"g{idx}", tag="sc_g")
                gv = g8[:]
                rview = bass.AP(gv.tensor, gv.offset,
                                [list(gv.ap[0]), [1, 2], [2, N_CORES]])
                nc.vector.tensor_reduce(g[:], rview, mybir.AxisListType.X, OP.add)
                return g

            def conv(pad, ws, i, stats, s_col, a_col):
                """3x3 conv of padded +/-1 fp8 image (pitch 64, duplicate copy
                at +PADX for the dh=2 pair) with 9 [C,C] taps; 2 groups of 4
                PSUM banks. Evacuation = single ACT Prelu from PSUM -> cf
                (scale s_col folded in when given); bn_stats read cf."""
                ng = 4
                padf = pad[:].rearrange("p two x -> p (two x)")
                wbase = ws[:, 0, :]
                for g in range(N_TILES // ng):
                    psg = psum.tile([C, ng, 512], F32, tag="ps",
                                    name=f"psg{g}", bufs=8 // ng)
                    fine = (i == N_LOC - 1 and g == 1)
                    tile_sets = ((0, 2), (2, 4)) if fine else ((0, ng),)

                    def emit_passes(t0, t1):
                        for dw in range(3):
                            wp = bass.AP(wbase.tensor, wbase.offset + dw * C,
                                         [list(wbase.ap[0]), [3 * C, 2], [1, C]])
                            for t in range(t0, t1):
                                q0 = (g * ng + t) * QSPAN + dw
                                rhs = bass.AP(padf.tensor, padf.offset + q0,
                                              [list(padf.ap[0]), [PITCH, 2],
                                               [1, QSPAN]])
                                nc.tensor.matmul(
                                    psg[:, t, 0:QSPAN], wp, rhs,
                                    start=(dw == 0), stop=False,
                                    perf_mode=mybir.MatmulPerfMode.DoubleRow,
                                )
                        if PAIR_DH2:
                            # taps (2,0)+(2,1) as one DR pair: second element
                            # from the duplicate pad copy at +PADX+1 (16-mult)
                            wp2 = bass.AP(wbase.tensor, wbase.offset + 6 * C,
                                          [list(wbase.ap[0]), [C, 2], [1, C]])
                            for t in range(t0, t1):
                                q0 = (g * ng + t) * QSPAN + 2 * PITCH
                                rhs = bass.AP(padf.tensor, padf.offset + q0,
                                              [list(padf.ap[0]), [PADX + 1, 2],
                                               [1, QSPAN]])
                                nc.tensor.matmul(
                                    psg[:, t, 0:QSPAN], wp2, rhs, start=False,
                                    stop=False,
                                    perf_mode=mybir.MatmulPerfMode.DoubleRow,
                                )
                            last_dws = (2,)
                        else:
                            last_dws = (0, 1, 2)
                        for li, dw in enumerate(last_dws):
                            for t in range(t0, t1):
                                q0 = (g * ng + t) * QSPAN + 2 * PITCH + dw
                                nc.tensor.matmul(
                                    psg[:, t, 0:QSPAN], ws[:, 6 + dw, :],
                                    padf[:, q0 : q0 + QSPAN],
                                    start=False,
                                    stop=(li == len(last_dws) - 1),
                                )

                    for t0, t1 in tile_sets:
                        emit_passes(t0, t1)
                    gbase = psg[:]

                    def sub_ap(t0, nt):
                        src = bass.AP(gbase.tensor, gbase.offset + t0 * 512,
                                      [list(gbase.ap[0]), [512, nt],
                                       [PITCH, TILE_ROWS], [1, W]])
                        d = cf[:, i, (g * ng + t0) * CHUNK
                               : (g * ng + t0 + nt) * CHUNK].rearrange(
                            "p (t r w) -> p t r w", r=TILE_ROWS, w=W)
                        return src, d

                    pieces = ((0, 2), (2, 2)) if fine else ((0, ng),)
                    for p0, nt in pieces:
                        src, d = sub_ap(p0, nt)
                        kw = {} if s_col is None else {"scale": s_col}
                        nc.scalar.activation(d, src, AF.Prelu,
                                             alpha=a_col, **kw)
                        for t in range(nt):
                            cch = i * 8 + g * ng + p0 + t
                            nc.vector.bn_stats(
                                stats[:, cch, :],
                                cf[:, i, (g * ng + p0 + t) * CHUNK
                                   : (g * ng + p0 + t + 1) * CHUNK])

            def sign_to_pad(pad, src_img, scale, bias):
                """ACT Sign(scale*src + bias) -> pad interior; then duplicate
                the padded span into the second copy (Pool engine, no DMA)."""
                pv = pad[:, 0, 0 : 59 * PITCH].rearrange(
                    "p (h w) -> p h w", h=59, w=PITCH)
                nc.scalar.activation(
                    pv[:, 1 : H + 1, 1 : W + 1], src_img,
                    AF.Sign, bias=bias, scale=scale)
                if PAIR_DH2:
                    nc.gpsimd.tensor_copy(pad[:, 1, 0 : 59 * PITCH],
                                          pad[:, 0, 0 : 59 * PITCH])

            for _rep in range(reps):
                # ============ Phase A: load x, BN1 stats, f16 residual ======
                QTR = HW // 4  # 784 = 2*CHUNK
                for i in range(N_LOC):
                    xin = work.tile([C, HW], F32, tag="x32", bufs=4)
                    nc.sync.dma_start(xin[:], x_d[i])
                    if i == 0 and _rep == 0:
                        load_params()
                        nc.vector.tensor_tensor(out=s1sq[:], in0=s1, in1=s1,
                                                op=OP.mult)
                    for q in range(8):
                        nc.vector.bn_stats(
                            stats1[:, i * 8 + q, :],
                            xin[:, q * CHUNK : (q + 1) * CHUNK])
                    for qq in range(4):
                        cvt = (nc.vector if qq == 3 else nc.gpsimd)
                        cvt.tensor_copy(
                            xf16[:, i, qq * QTR : (qq + 1) * QTR],
                            xin[:, qq * QTR : (qq + 1) * QTR])

                if _rep == 0:
                    # zero only the border cells of half 0: interior is
                    # rewritten per image, half 1 is refreshed by the copies
                    for p in pads:
                        nc.vector.memset(p[:, 0, 0:PITCH], 0.0)  # top row
                        nc.vector.memset(
                            p[:, 0, 57 * PITCH : 59 * PITCH], 0.0)  # bottom
                        pv = p[:, 0, 0 : 59 * PITCH]
                        col = bass.AP(pv.tensor, pv.offset + PITCH,
                                      [list(pv.ap[0]), [PITCH, 56], [1, 1]])
                        nc.vector.memset(col, 0.0)  # left pad col
                        colr = bass.AP(pv.tensor, pv.offset + PITCH + W + 1,
                                       [list(pv.ap[0]), [PITCH, 56], [1, 7]])
                        nc.vector.memset(colr, 0.0)  # right pad cols

                g1ar = reduce_stats(stats1, 0)
                mean1, rstd1 = _rstd_from_allreduced(nc, const, g1ar, "1")
                k1, c1b = _affine_consts(nc, const, pp, mean1, rstd1, P_G1, P_B1, "1")

                # ============ Phase B: sign(BN1(x)); conv1 -> q1; stats2 ====
                # sign/copy for image i+1 are issued before conv(i) so the
                # ACT queue never stalls PE on the next image's pad
                def sign1(i):
                    pad = pads[i % 2]
                    xin = work.tile([C, HW], F32, tag="x32", bufs=4)
                    if i < 2:
                        # quarter DMAs: the boundary's tiny cci DMA is not
                        # stuck behind a 4.5us transfer on the DMA engines
                        for qq in range(4):
                            nc.sync.dma_start(
                                xin[:, qq * QTR : (qq + 1) * QTR],
                                x_d[i, :, qq * QTR : (qq + 1) * QTR])
                    else:
                        nc.sync.dma_start(xin[:], x_d[i])
                    pv = pad[:, 0, 0 : 59 * PITCH].rearrange(
                        "p (h w) -> p h w", h=59, w=PITCH)
                    # image 0 (phase ramp): small first slice so PE starts
                    # sooner + split copies; steady images: single pass
                    slices = ((0, 8), (8, 30), (30, 56)) if i == 0 \
                        else ((0, 56),)
                    for r0, r1 in slices:
                        nc.scalar.activation(
                            pv[:, 1 + r0 : 1 + r1, 1 : W + 1],
                            xin[:, r0 * W : r1 * W].rearrange(
                                "p (h w) -> p h w", h=r1 - r0, w=W),
                            AF.Sign, bias=c1b[:], scale=k1[:])
                        if PAIR_DH2 and r1 >= 30:
                            if r1 == 56 and r0 == 0:
                                for lo, hi in ((0, 31 * PITCH),
                                               (31 * PITCH, 59 * PITCH)):
                                    nc.gpsimd.tensor_copy(pad[:, 1, lo:hi],
                                                          pad[:, 0, lo:hi])
                            else:
                                lo, hi = (0, 31 * PITCH) if r1 == 30 else \
                                         (31 * PITCH, 59 * PITCH)
                                nc.gpsimd.tensor_copy(pad[:, 1, lo:hi],
                                                      pad[:, 0, lo:hi])

                sign1(0)
                for i in range(N_LOC):
                    if i + 1 < N_LOC:
                        sign1(i + 1)
                    conv(pads[i % 2], w1s, i, stats2, None, a1)

                g2ar = reduce_stats(stats2, 1, s1, s1sq[:])
                mean2, rstd2 = _rstd_from_allreduced(nc, const, g2ar, "2")
                k2, c2b = _affine_consts(nc, const, pp, mean2, rstd2, P_G2, P_B2, "2")
                # sign2 = Sign(k2*p1 + cb2) = Sign((k2*s1)*q1 + cb2)
                ks2 = const.tile([C, 1], F32, name="ks2")
                nc.vector.tensor_tensor(out=ks2[:], in0=k2[:], in1=s1, op=OP.mult)

                # ============ Phase C: sign2(q1); conv2 -> p2; stats3 =======
                def sign2(i):
                    pad = pads[i % 2]
                    pv = pad[:, 0, 0 : 59 * PITCH].rearrange(
                        "p (h w) -> p h w", h=59, w=PITCH)
                    slices = ((0, 8), (8, 30), (30, 56)) if i == 0 \
                        else ((0, 56),)
                    for r0, r1 in slices:
                        nc.scalar.activation(
                            pv[:, 1 + r0 : 1 + r1, 1 : W + 1],
                            cf[:, i, r0 * W : r1 * W].rearrange(
                                "p (h w) -> p h w", h=r1 - r0, w=W),
                            AF.Sign, bias=c2b[:], scale=ks2[:])
                        if PAIR_DH2 and r1 >= 30:
                            if r1 == 56 and r0 == 0:
                                for lo, hi in ((0, 31 * PITCH),
                                               (31 * PITCH, 59 * PITCH)):
                                    nc.gpsimd.tensor_copy(pad[:, 1, lo:hi],
                                                          pad[:, 0, lo:hi])
                            else:
                                lo, hi = (0, 31 * PITCH) if r1 == 30 else \
                                         (31 * PITCH, 59 * PITCH)
                                nc.gpsimd.tensor_copy(pad[:, 1, lo:hi],
                                                      pad[:, 0, lo:hi])

                sign2(0)
                for i in range(N_LOC):
                    if i + 1 < N_LOC:
                        sign2(i + 1)
                    conv(pads[i % 2], w2s, i, stats3, s2, a2)

                g3ar = reduce_stats(stats3, 2)
                mean3, rstd3 = _rstd_from_allreduced(nc, const, g3ar, "3")
                k3, c3b = _affine_consts(nc, const, pp, mean3, rstd3, P_G3, P_B3, "3")

                # ====== Phase D: y = PReLU(k3*p2 + x + c3b) =================
                # z = k3*p2 + x in ONE stt; c3b folds into the prelu bias
                for i in range(N_LOC):
                    for hh in range(2):
                        sl = slice(hh * HALF, (hh + 1) * HALF)
                        h = work.tile([C, HALF], F16, tag="h16", bufs=3)
                        nc.vector.tensor_scalar(h[:], cf[:, i, sl],
                                                k3[:], c3b[:], OP.mult, OP.add)
                        z = work.tile([C, HALF], F16, tag="z16", bufs=4)
                        nc.vector.tensor_tensor(out=z[:], in0=h[:],
                                                in1=xf16[:, i, sl], op=OP.add)
                        y = work.tile([C, HALF], F16, tag="y16", bufs=4)
                        nc.scalar.activation(y[:], z[:], AF.Prelu,
                                             alpha=a3[:])
                        nc.sync.dma_start(out_d[i, :, sl], y[:])

    nc.compile()
    return nc


def _prep_host(x, bn1_g, bn1_b, w1, prelu1_a, bn2_g, bn2_b, w2, prelu2_a,
               bn3_g, bn3_b, prelu3_a):
    def wprep(w_flat):
        w = np.asarray(w_flat, np.float32).reshape(C, C, 3, 3)
        scale = np.mean(np.abs(w), axis=(1, 2, 3)).astype(np.float32)  # [C]
        # lhsT layout [tap, i, o] = sign(w[o, i, dh, dw])
        wT = np.sign(w).transpose(2, 3, 1, 0).reshape(9, C, C)
        return wT.astype(mybir.dt.np(FP8)), scale

    w1t, s1 = wprep(w1)
    w2t, s2 = wprep(w2)

    pp = np.zeros((C, NP), np.float32)
    pp[:, P_S1] = s1
    pp[:, P_S2] = s2
    pp[:, P_G1] = np.asarray(bn1_g, np.float32)
    pp[:, P_B1] = np.asarray(bn1_b, np.float32)
    pp[:, P_G2] = np.asarray(bn2_g, np.float32)
    pp[:, P_B2] = np.asarray(bn2_b, np.float32)
    pp[:, P_G3] = np.asarray(bn3_g, np.float32)
    pp[:, P_B3] = np.asarray(bn3_b, np.float32)
    pp[:, P_A1] = np.float32(prelu1_a)
    pp[:, P_A2] = np.float32(prelu2_a)
    pp[:, P_A3] = np.float32(prelu3_a)

    x = np.ascontiguousarray(np.asarray(x, np.float32).reshape(64, C, HW))
    in_maps = []
    for r in range(N_CORES):
        in_maps.append({
            "x": x[r * N_LOC : (r + 1) * N_LOC],
            "w1t": w1t,
            "w2t": w2t,
            "pp": pp,
        })
    return in_maps


_NC_CACHE = None


def _get_nc():
    global _NC_CACHE
    if _NC_CACHE is None:
        _NC_CACHE = build_nc()
    return _NC_CACHE


def run(in_maps, **kwargs):
    nc = _get_nc()
    return run_bass_kernel_spmd(nc, in_maps, core_ids=list(range(N_CORES)), **kwargs)


def kernel(**inputs):
    in_maps = _prep_host(**inputs)
    last_err = None
    for attempt in range(3):
        try:
            res = run(in_maps)
            break
        except Exception as e:  # transient NRT device errors happen; retry
            last_err = e
            import time as _time
            _time.sleep(2.0)
    else:
        raise last_err
    out = np.concatenate(
        [np.asarray(r["out"]).astype(np.float32).reshape(N_LOC, C, H, W)
         for r in res.results], axis=0
    )
    return out


if __name__ == "__main__":
    rng = np.random.default_rng(0)
    x = rng.standard_normal((64, C, H, W), dtype=np.float32)
    w1 = ((rng.random((C * C * 9, 1), dtype=np.float32) - 0.5) * 0.002)
    w2 = ((rng.random((C * C * 9, 1), dtype=np.float32) - 0.5) * 0.002)
    ones = np.ones(C, np.float32)
    zeros = np.zeros(C, np.float32)
    y = kernel(x=x, bn1_g=ones, bn1_b=zeros, w1=w1, prelu1_a=np.float32(0.25),
               bn2_g=ones, bn2_b=zeros, w2=w2, prelu2_a=np.float32(0.25),
               bn3_g=ones, bn3_b=zeros, prelu3_a=np.float32(0.25))
    print("out", y.shape, y.dtype, float(np.abs(y).mean()))


# revision 49
# speedup vs baseline: 1.2729x; 1.0118x over previous
"""XNOR-Net BasicBlock forward (BN-sign-binconv-PReLU x2 + BN + residual + PReLU)
distributed over 8 Trainium2 NeuronCores, data-parallel over the batch axis.

Self-contained: hardcodes shapes N=64, C=128, H=W=56, 8 cores.
"""

import numpy as np
import ml_dtypes

import concourse.bass as bass
import concourse.mybir as mybir
import concourse.tile as tile
from concourse import bacc
from concourse.bass_utils import run_bass_kernel_spmd

F32 = mybir.dt.float32
F16 = mybir.dt.float16
BF16 = mybir.dt.bfloat16
FP8 = mybir.dt.float8e4
PITCH = 64
AF = mybir.ActivationFunctionType
OP = mybir.AluOpType

N_CORES = 8
N_LOC = 8          # images per core
C = 128            # channels (== partitions)
H = W = 56
HW = H * W         # 3136
EPS = 1e-5
TILE_ROWS = 7      # output rows per PSUM bank span: 7*64 = 448 <= 512
N_TILES = H // TILE_ROWS   # 8 tiles -> 2 groups of 4 banks
QSPAN = TILE_ROWS * PITCH  # 448
CHUNK = TILE_ROWS * W      # 392
HALF = HW // 2             # 1568
# padded image: 59 rows x 64 pitch = 3776; second copy at +PADX where
# (PADX + 1) % 16 == 0 so the dh=2 horizontal tap pair is DoubleRow-legal
PADX = 3791
PAIR_DH2 = True

# pp param columns
P_S1, P_S2, P_G1, P_B1, P_G2, P_B2, P_G3, P_B3, P_A1, P_A2, P_A3 = range(11)
NP = 11


def _col(t, j):
    return t[:, j : j + 1]


def _rstd_from_allreduced(nc, pool, ar, name):
    """ar: [128,2] = sum over cores of [mean_i, var_i + mean_i^2].
    Returns (mean, rstd) tiles [128,1] f32 with rstd = 1/sqrt(var+EPS),
    Newton-refined to cover ScalarE Sqrt spline error."""
    mean = pool.tile([C, 1], F32, name=f"mean_{name}", tag=f"mean_{name}")
    ex2 = pool.tile([C, 1], F32, name=f"ex2_{name}", tag="sc_ex2")
    nc.vector.tensor_scalar_mul(mean[:], _col(ar, 0), 1.0 / N_CORES)
    nc.vector.tensor_scalar_mul(ex2[:], _col(ar, 1), 1.0 / N_CORES)
    negmean = pool.tile([C, 1], F32, name=f"negmean_{name}", tag="sc_negmean")
    nc.vector.tensor_scalar_mul(negmean[:], mean[:], -1.0)
    vpe = pool.tile([C, 1], F32, name=f"vpe_{name}", tag="sc_vpe")
    nc.vector.scalar_tensor_tensor(vpe[:], mean[:], negmean[:], ex2[:], OP.mult, OP.add)
    nc.vector.tensor_scalar_add(vpe[:], vpe[:], EPS)
    rec = pool.tile([C, 1], F32, name=f"rec_{name}", tag="sc_rec")
    nc.vector.reciprocal(rec[:], vpe[:])
    rstd = pool.tile([C, 1], F32, name=f"rstd_{name}", tag=f"rstd_{name}")
    nc.scalar.activation(rstd[:], rec[:], AF.Sqrt)
    # Newton: y <- y * (1.5 - 0.5 * vpe * y^2)
    t1 = pool.tile([C, 1], F32, name=f"t1_{name}", tag="sc_t1")
    nc.vector.tensor_tensor(out=t1[:], in0=rstd[:], in1=rstd[:], op=OP.mult)
    nc.vector.tensor_tensor(out=t1[:], in0=t1[:], in1=vpe[:], op=OP.mult)
    nc.vector.tensor_scalar(t1[:], t1[:], -0.5, 1.5, OP.mult, OP.add)
    nc.vector.tensor_tensor(out=rstd[:], in0=rstd[:], in1=t1[:], op=OP.mult)
    return mean, rstd


def _affine_consts(nc, pool, pp, mean, rstd, g_col, b_col, name):
    """k = g * rstd ; cb = b - mean * k. Returns (k, cb) tiles [128,1]."""
    k = pool.tile([C, 1], F32, name=f"k_{name}", tag=f"k_{name}")
    nc.vector.tensor_tensor(out=k[:], in0=_col(pp, g_col), in1=rstd[:], op=OP.mult)
    negk = pool.tile([C, 1], F32, name=f"negk_{name}", tag="sc_negk")
    nc.vector.tensor_scalar_mul(negk[:], k[:], -1.0)
    cb = pool.tile([C, 1], F32, name=f"cb_{name}", tag=f"cb_{name}")
    nc.vector.scalar_tensor_tensor(
        cb[:], mean[:], negk[:], _col(pp, b_col), OP.mult, OP.add
    )
    return k, cb


def _sign_threshold(nc, pool, k, cb, ra, rs, name):
    """b = sign(k*prelu(s*c) + cb) == Sign(c*sgn - sgn*tau) for monotone prelu
    (a>0). ra=1/a, rs=1/s precomputed. Returns (sgn, nbias) [128,1] tiles."""
    negcb = pool.tile([C, 1], F32, name=f"negcb_{name}", tag="sc_negcb")
    nc.vector.tensor_scalar_mul(negcb[:], cb[:], -1.0)
    rk = pool.tile([C, 1], F32, name=f"rk_{name}", tag="sc_rk")
    nc.vector.reciprocal(rk[:], k[:])
    t2 = pool.tile([C, 1], F32, name=f"t2_{name}", tag="sc_t2")
    nc.vector.tensor_tensor(out=t2[:], in0=negcb[:], in1=rk[:], op=OP.mult)
    # prelu^-1(t2) = max(t2,0) + min(t2,0)/a
    tpos = pool.tile([C, 1], F32, name=f"tpos_{name}", tag="sc_tpos")
    nc.vector.tensor_scalar_max(tpos[:], t2[:], 0.0)
    tneg = pool.tile([C, 1], F32, name=f"tneg_{name}", tag="sc_tneg")
    nc.vector.tensor_scalar_min(tneg[:], t2[:], 0.0)
    pinv = pool.tile([C, 1], F32, name=f"pinv_{name}", tag="sc_pinv")
    nc.vector.scalar_tensor_tensor(pinv[:], tneg[:], ra[:], tpos[:],
                                   OP.mult, OP.add)
    tau = pool.tile([C, 1], F32, name=f"tau_{name}", tag="sc_tau")
    nc.vector.tensor_tensor(out=tau[:], in0=pinv[:], in1=rs[:], op=OP.mult)
    sgn = pool.tile([C, 1], F32, name=f"sgn_{name}", tag=f"sgn_{name}")
    nc.scalar.activation(sgn[:], k[:], AF.Sign)
    nbias = pool.tile([C, 1], F32, name=f"nbias_{name}", tag=f"nbias_{name}")
    nc.vector.tensor_tensor(out=nbias[:], in0=sgn[:], in1=tau[:], op=OP.mult)
    nc.vector.tensor_scalar_mul(nbias[:], nbias[:], -1.0)
    return sgn, nbias


def build_nc(reps=1):
    nc = bacc.Bacc(None, target_bir_lowering=False, debug=False, num_devices=N_CORES)

    x_d = nc.dram_tensor("x", [N_LOC, C, HW], F32, kind="ExternalInput")
    w1_d = nc.dram_tensor("w1t", [9, C, C], FP8, kind="ExternalInput")
    w2_d = nc.dram_tensor("w2t", [9, C, C], FP8, kind="ExternalInput")
    pp_d = nc.dram_tensor("pp", [C, NP], F32, kind="ExternalInput")
    out_d = nc.dram_tensor("out", [N_LOC, C, HW], F16, kind="ExternalOutput")

    with tile.TileContext(nc) as tc:
        with (
            tc.tile_pool(name="const", bufs=1) as const,
            tc.tile_pool(name="work", bufs=2) as work,
            tc.tile_pool(name="psum", bufs=2, space="PSUM") as psum,
            tc.tile_pool(name="dram", bufs=1, space="DRAM") as dram,
        ):
            # ---- persistent SBUF tensors (loads issued after the first
            # x image so phase A's DMA stream starts immediately) ----
            pp = const.tile([C, NP], F32)
            w1s = const.tile([C, 9, C], FP8)
            w2s = const.tile([C, 9, C], FP8)

            def load_params():
                nc.gpsimd.dma_start(pp[:], pp_d[:])
                for ws, wd in ((w1s, w1_d), (w2s, w2_d)):
                    wv = wd[:]
                    srcw = bass.AP(wv.tensor, wv.offset,
                                   [[C, C], [C * C, 9], [1, C]])
                    nc.gpsimd.dma_start(ws[:], srcw)
            xf16 = const.tile([C, N_LOC, HW], F16)   # residual copy of x
            # one shared buffer: phase B writes q1 = prelu(conv1) (EXACT in
            # f16: conv1 is even ints <=1152, a=0.25 a power of two); phase C
            # overwrites image i with p2 = prelu(s2*conv2) after sign2(i)
            # consumed it (ACT program order guarantees the WAR ordering)
            cf = const.tile([C, N_LOC, HW], F16)
            stats1 = const.tile([C, N_LOC * 8, 6], F32, tag="st1")
            stats2 = const.tile([C, N_LOC * 8, 6], F32, tag="st2")
            stats3 = const.tile([C, N_LOC * 8, 6], F32, tag="st3")
            # pads memset-ed after phase A (runs during collective-1 idle)
            pads = [const.tile([C, 2, PADX], FP8, name=f"pad{j}")
                    for j in range(2)]

            a1 = _col(pp, P_A1)
            a2 = _col(pp, P_A2)
            a3 = _col(pp, P_A3)
            s1 = _col(pp, P_S1)
            s2 = _col(pp, P_S2)

            s1sq = const.tile([C, 1], F32, name="s1sq")
            s2sq = const.tile([C, 1], F32, name="s2sq")

            cc_counter = [0]

            def reduce_stats(stats, idx, s_col=None, s2_col=None):
                """bn_aggr + pack [mean, var+mean^2] (optionally rescaled
                from q- to p-domain by s / s^2) + allgather + local reduce;
                returns [128,2] tile of cross-core sums."""
                mv = const.tile([C, 2], F32, name=f"mv{idx}", tag="sc_mv")
                nc.vector.bn_aggr(mv[:], stats[:])
                e = const.tile([C, 2], F32, name=f"e{idx}", tag="sc_e")
                nc.vector.scalar_tensor_tensor(
                    _col(e, 1), _col(mv, 0), _col(mv, 0), _col(mv, 1), OP.mult, OP.add
                )
                if s_col is not None:
                    nc.vector.tensor_tensor(out=_col(e, 0), in0=_col(mv, 0),
                                            in1=s_col, op=OP.mult)
                    nc.vector.tensor_tensor(out=_col(e, 1), in0=_col(e, 1),
                                            in1=s2_col, op=OP.mult)
                else:
                    nc.vector.tensor_copy(_col(e, 0), _col(mv, 0))
                n = cc_counter[0]
                cc_counter[0] += 1
                cci = dram.tile([C, 2], F32, name=f"cc_in{n}", tag=f"cc_in{n}")
                cco = dram.tile([N_CORES, C, 2], F32, name=f"cc_out{n}",
                                tag=f"cc_out{n}", addr_space="Shared")
                nc.sync.dma_start(cci[:], e[:])
                nc.gpsimd.collective_compute(
                    "AllGather",
                    OP.bypass,
                    replica_groups=[list(range(N_CORES))],
                    ins=[cci.opt()],
                    outs=[cco.opt()],
                )
                g8 = const.tile([C, N_CORES, 2], F32, name=f"g8{idx}", tag="sc_g8")
                cav = cco[:]  # AP over [8, C, 2] dram tensor
                src = bass.AP(cav.tensor, cav.offset,
                              [[2, C], [2 * C, N_CORES], [1, 2]])
                nc.sync.dma_start(g8[:], src)  # contiguous 8B runs
                g = const.tile([C, 2], F32, name=f"g{idx}", tag="sc_g")
                gv = g8[:]
                rview = bass.AP(gv.tensor, gv.offset,
                                [list(gv.ap[0]), [1, 2], [2, N_CORES]])
                nc.vector.tensor_reduce(g[:], rview, mybir.AxisListType.X, OP.add)
                return g

            def conv(pad, ws, i, stats, s_col, a_col):
                """3x3 conv of padded +/-1 fp8 image (pitch 64, duplicate copy
                at +PADX for the dh=2 pair) with 9 [C,C] taps; 2 groups of 4
                PSUM banks. Evacuation = single ACT Prelu from PSUM -> cf
                (scale s_col folded in when given); bn_stats read cf."""
                ng = 4
                padf = pad[:].rearrange("p two x -> p (two x)")
                wbase = ws[:, 0, :]
                for g in range(N_TILES // ng):
                    psg = psum.tile([C, ng, 512], F32, tag="ps",
                                    name=f"psg{g}", bufs=8 // ng)
                    fine = (i == N_LOC - 1 and g == 1)
                    tile_sets = ((0, 2), (2, 4)) if fine else ((0, ng),)

                    def emit_passes(t0, t1):
                        for dw in range(3):
                            wp = bass.AP(wbase.tensor, wbase.offset + dw * C,
                                         [list(wbase.ap[0]), [3 * C, 2], [1, C]])
                            for t in range(t0, t1):
                                q0 = (g * ng + t) * QSPAN + dw
                                rhs = bass.AP(padf.tensor, padf.offset + q0,
                                              [list(padf.ap[0]), [PITCH, 2],
                                               [1, QSPAN]])
                                nc.tensor.matmul(
                                    psg[:, t, 0:QSPAN], wp, rhs,
                                    start=(dw == 0), stop=False,
                                    perf_mode=mybir.MatmulPerfMode.DoubleRow,
                                )
                        if PAIR_DH2:
                            # taps (2,0)+(2,1) as one DR pair: second element
                            # from the duplicate pad copy at +PADX+1 (16-mult)
                            wp2 = bass.AP(wbase.tensor, wbase.offset + 6 * C,
                                          [list(wbase.ap[0]), [C, 2], [1, C]])
                            for t in range(t0, t1):
                                q0 = (g * ng + t) * QSPAN + 2 * PITCH
                                rhs = bass.AP(padf.tensor, padf.offset + q0,
                                              [list(padf.ap[0]), [PADX + 1, 2],
                                               [1, QSPAN]])
                                nc.tensor.matmul(
                                    psg[:, t, 0:QSPAN], wp2, rhs, start=False,
                                    stop=False,
                                    perf_mode=mybir.MatmulPerfMode.DoubleRow,
                                )
                            last_dws = (2,)
                        else:
                            last_dws = (0, 1, 2)
                        for li, dw in enumerate(last_dws):
                            for t in range(t0, t1):
                                q0 = (g * ng + t) * QSPAN + 2 * PITCH + dw
                                nc.tensor.matmul(
                                    psg[:, t, 0:QSPAN], ws[:, 6 + dw, :],
                                    padf[:, q0 : q0 + QSPAN],
                                    start=False,
                                    stop=(li == len(last_dws) - 1),
                                )

                    for t0, t1 in tile_sets:
                        emit_passes(t0, t1)
                    gbase = psg[:]

                    def sub_ap(t0, nt):
                        src = bass.AP(gbase.tensor, gbase.offset + t0 * 512,
                                      [list(gbase.ap[0]), [512, nt],
                                       [PITCH, TILE_ROWS], [1, W]])
                        d = cf[:, i, (g * ng + t0) * CHUNK
                               : (g * ng + t0 + nt) * CHUNK].rearrange(
                            "p (t r w) -> p t r w", r=TILE_ROWS, w=W)
                        return src, d

                    pieces = ((0, 2), (2, 2)) if fine else ((0, ng),)
                    for p0, nt in pieces:
                        src, d = sub_ap(p0, nt)
                        nc.scalar.activation(d, src, AF.Prelu,
                                             alpha=a_col)
                        for t in range(nt):
                            cch = i * 8 + g * ng + p0 + t
                            nc.vector.bn_stats(
                                stats[:, cch, :],
                                cf[:, i, (g * ng + p0 + t) * CHUNK
                                   : (g * ng + p0 + t + 1) * CHUNK])

            def sign_to_pad(pad, src_img, scale, bias):
                """ACT Sign(scale*src + bias) -> pad interior; then duplicate
                the padded span into the second copy (Pool engine, no DMA)."""
                pv = pad[:, 0, 0 : 59 * PITCH].rearrange(
                    "p (h w) -> p h w", h=59, w=PITCH)
                nc.scalar.activation(
                    pv[:, 1 : H + 1, 1 : W + 1], src_img,
                    AF.Sign, bias=bias, scale=scale)
                if PAIR_DH2:
                    nc.gpsimd.tensor_copy(pad[:, 1, 0 : 59 * PITCH],
                                          pad[:, 0, 0 : 59 * PITCH])

            for _rep in range(reps):
                # ============ Phase A: load x, BN1 stats, f16 residual ======
                QTR = HW // 4  # 784 = 2*CHUNK
                for i in range(N_LOC):
                    xin = work.tile([C, HW], F32, tag="x32", bufs=4)
                    if i == N_LOC - 1:
                        for hh in range(2):
                            nc.sync.dma_start(
                                xin[:, hh * HALF : (hh + 1) * HALF],
                                x_d[i, :, hh * HALF : (hh + 1) * HALF])
                    else:
                        nc.sync.dma_start(xin[:], x_d[i])
                    if i == 0 and _rep == 0:
                        load_params()
                        nc.vector.tensor_tensor(out=s1sq[:], in0=s1, in1=s1,
                                                op=OP.mult)
                        nc.vector.tensor_tensor(out=s2sq[:], in0=s2, in1=s2,
                                                op=OP.mult)
                    for q in range(8):
                        nc.vector.bn_stats(
                            stats1[:, i * 8 + q, :],
                            xin[:, q * CHUNK : (q + 1) * CHUNK])
                    for qq in range(4):
                        cvt = (nc.vector if qq == 3 else nc.gpsimd)
                        cvt.tensor_copy(
                            xf16[:, i, qq * QTR : (qq + 1) * QTR],
                            xin[:, qq * QTR : (qq + 1) * QTR])

                if _rep == 0:
                    # zero only the border cells of half 0: interior is
                    # rewritten per image, half 1 is refreshed by the copies
                    for p in pads:
                        nc.vector.memset(p[:, 0, 0:PITCH], 0.0)  # top row
                        nc.vector.memset(
                            p[:, 0, 57 * PITCH : 59 * PITCH], 0.0)  # bottom
                        pv = p[:, 0, 0 : 59 * PITCH]
                        col = bass.AP(pv.tensor, pv.offset + PITCH,
                                      [list(pv.ap[0]), [PITCH, 56], [1, 1]])
                        nc.vector.memset(col, 0.0)  # left pad col
                        colr = bass.AP(pv.tensor, pv.offset + PITCH + W + 1,
                                       [list(pv.ap[0]), [PITCH, 56], [1, 7]])
                        nc.vector.memset(colr, 0.0)  # right pad cols

                g1ar = reduce_stats(stats1, 0)
                mean1, rstd1 = _rstd_from_allreduced(nc, const, g1ar, "1")
                k1, c1b = _affine_consts(nc, const, pp, mean1, rstd1, P_G1, P_B1, "1")

                # ============ Phase B: sign(BN1(x)); conv1 -> q1; stats2 ====
                # sign/copy for image i+1 are issued before conv(i) so the
                # ACT queue never stalls PE on the next image's pad
                def sign1(i):
                    pad = pads[i % 2]
                    xin = work.tile([C, HW], F32, tag="x32", bufs=4)
                    if i < 2:
                        # quarter DMAs: the boundary's tiny cci DMA is not
                        # stuck behind a 4.5us transfer on the DMA engines
                        for qq in range(4):
                            nc.sync.dma_start(
                                xin[:, qq * QTR : (qq + 1) * QTR],
                                x_d[i, :, qq * QTR : (qq + 1) * QTR])
                    else:
                        nc.sync.dma_start(xin[:], x_d[i])
                    pv = pad[:, 0, 0 : 59 * PITCH].rearrange(
                        "p (h w) -> p h w", h=59, w=PITCH)
                    # image 0 (phase ramp): small first slice so PE starts
                    # sooner + split copies; steady images: single pass
                    slices = ((0, 8), (8, 30), (30, 56)) if i == 0 \
                        else ((0, 56),)
                    for r0, r1 in slices:
                        nc.scalar.activation(
                            pv[:, 1 + r0 : 1 + r1, 1 : W + 1],
                            xin[:, r0 * W : r1 * W].rearrange(
                                "p (h w) -> p h w", h=r1 - r0, w=W),
                            AF.Sign, bias=c1b[:], scale=k1[:])
                        if PAIR_DH2 and r1 >= 30:
                            if r1 == 56 and r0 == 0:
                                for lo, hi in ((0, 31 * PITCH),
                                               (31 * PITCH, 59 * PITCH)):
                                    nc.gpsimd.tensor_copy(pad[:, 1, lo:hi],
                                                          pad[:, 0, lo:hi])
                            else:
                                lo, hi = (0, 31 * PITCH) if r1 == 30 else \
                                         (31 * PITCH, 59 * PITCH)
                                nc.gpsimd.tensor_copy(pad[:, 1, lo:hi],
                                                      pad[:, 0, lo:hi])

                sign1(0)
                for i in range(N_LOC):
                    if i + 1 < N_LOC:
                        sign1(i + 1)
                    conv(pads[i % 2], w1s, i, stats2, None, a1)

                g2ar = reduce_stats(stats2, 1, s1, s1sq[:])
                mean2, rstd2 = _rstd_from_allreduced(nc, const, g2ar, "2")
                k2, c2b = _affine_consts(nc, const, pp, mean2, rstd2, P_G2, P_B2, "2")
                # sign2 = Sign(k2*p1 + cb2) = Sign((k2*s1)*q1 + cb2)
                ks2 = const.tile([C, 1], F32, name="ks2")
                nc.vector.tensor_tensor(out=ks2[:], in0=k2[:], in1=s1, op=OP.mult)

                # ============ Phase C: sign2(q1); conv2 -> p2; stats3 =======
                def sign2(i):
                    pad = pads[i % 2]
                    pv = pad[:, 0, 0 : 59 * PITCH].rearrange(
                        "p (h w) -> p h w", h=59, w=PITCH)
                    slices = ((0, 8), (8, 30), (30, 56)) if i == 0 \
                        else ((0, 56),)
                    for r0, r1 in slices:
                        nc.scalar.activation(
                            pv[:, 1 + r0 : 1 + r1, 1 : W + 1],
                            cf[:, i, r0 * W : r1 * W].rearrange(
                                "p (h w) -> p h w", h=r1 - r0, w=W),
                            AF.Sign, bias=c2b[:], scale=ks2[:])
                        if PAIR_DH2 and r1 >= 30:
                            if r1 == 56 and r0 == 0:
                                for lo, hi in ((0, 31 * PITCH),
                                               (31 * PITCH, 59 * PITCH)):
                                    nc.gpsimd.tensor_copy(pad[:, 1, lo:hi],
                                                          pad[:, 0, lo:hi])
                            else:
                                lo, hi = (0, 31 * PITCH) if r1 == 30 else \
                                         (31 * PITCH, 59 * PITCH)
                                nc.gpsimd.tensor_copy(pad[:, 1, lo:hi],
                                                      pad[:, 0, lo:hi])

                sign2(0)
                for i in range(N_LOC):
                    if i + 1 < N_LOC:
                        sign2(i + 1)
                    conv(pads[i % 2], w2s, i, stats3, None, a2)

                g3ar = reduce_stats(stats3, 2, s2, s2sq[:])
                mean3, rstd3 = _rstd_from_allreduced(nc, const, g3ar, "3")
                k3, c3b = _affine_consts(nc, const, pp, mean3, rstd3, P_G3, P_B3, "3")
                # cf holds q2; fold s2 into the phase-D scale: k3*p2 = (k3*s2)*q2
                ks3 = const.tile([C, 1], F32, name="ks3")
                nc.vector.tensor_tensor(out=ks3[:], in0=k3[:], in1=s2, op=OP.mult)

                # ====== Phase D: y = PReLU(k3*p2 + x + c3b) =================
                # z = k3*p2 + x in ONE stt; c3b folds into the prelu bias
                for i in range(N_LOC):
                    for hh in range(2):
                        sl = slice(hh * HALF, (hh + 1) * HALF)
                        h = work.tile([C, HALF], F16, tag="h16", bufs=3)
                        nc.vector.tensor_scalar(h[:], cf[:, i, sl],
                                                ks3[:], c3b[:], OP.mult, OP.add)
                        z = work.tile([C, HALF], F16, tag="z16", bufs=4)
                        nc.vector.tensor_tensor(out=z[:], in0=h[:],
                                                in1=xf16[:, i, sl], op=OP.add)
                        y = work.tile([C, HALF], F16, tag="y16", bufs=4)
                        if i == N_LOC - 1 and hh == 1:
                            # last half: prelu on DVE so ACT is not the tail
                            v = work.tile([C, HALF], F16, tag="h16", bufs=3)
                            nc.vector.tensor_scalar(v[:], z[:], a3[:], 0.0,
                                                    OP.mult, OP.add)
                            nc.vector.tensor_tensor(out=y[:], in0=z[:],
                                                    in1=v[:], op=OP.max)
                        else:
                            nc.scalar.activation(y[:], z[:], AF.Prelu,
                                                 alpha=a3[:])
                        nc.sync.dma_start(out_d[i, :, sl], y[:])

    nc.compile()
    return nc


def _prep_host(x, bn1_g, bn1_b, w1, prelu1_a, bn2_g, bn2_b, w2, prelu2_a,
               bn3_g, bn3_b, prelu3_a):
    def wprep(w_flat):
        w = np.asarray(w_flat, np.float32).reshape(C, C, 3, 3)
        scale = np.mean(np.abs(w), axis=(1, 2, 3)).astype(np.float32)  # [C]
        # lhsT layout [tap, i, o] = sign(w[o, i, dh, dw])
        wT = np.sign(w).transpose(2, 3, 1, 0).reshape(9, C, C)
        return wT.astype(mybir.dt.np(FP8)), scale

    w1t, s1 = wprep(w1)
    w2t, s2 = wprep(w2)

    pp = np.zeros((C, NP), np.float32)
    pp[:, P_S1] = s1
    pp[:, P_S2] = s2
    pp[:, P_G1] = np.asarray(bn1_g, np.float32)
    pp[:, P_B1] = np.asarray(bn1_b, np.float32)
    pp[:, P_G2] = np.asarray(bn2_g, np.float32)
    pp[:, P_B2] = np.asarray(bn2_b, np.float32)
    pp[:, P_G3] = np.asarray(bn3_g, np.float32)
    pp[:, P_B3] = np.asarray(bn3_b, np.float32)
    pp[:, P_A1] = np.float32(prelu1_a)
    pp[:, P_A2] = np.float32(prelu2_a)
    pp[:, P_A3] = np.float32(prelu3_a)

    x = np.ascontiguousarray(np.asarray(x, np.float32).reshape(64, C, HW))
    in_maps = []
    for r in range(N_CORES):
        in_maps.append({
            "x": x[r * N_LOC : (r + 1) * N_LOC],
            "w1t": w1t,
            "w2t": w2t,
            "pp": pp,
        })
    return in_maps


_NC_CACHE = None


def _get_nc():
    global _NC_CACHE
    if _NC_CACHE is None:
        _NC_CACHE = build_nc()
    return _NC_CACHE


def run(in_maps, **kwargs):
    nc = _get_nc()
    return run_bass_kernel_spmd(nc, in_maps, core_ids=list(range(N_CORES)), **kwargs)


def kernel(**inputs):
    in_maps = _prep_host(**inputs)
    last_err = None
    for attempt in range(3):
        try:
            res = run(in_maps)
            break
        except Exception as e:  # transient NRT device errors happen; retry
            last_err = e
            import time as _time
            _time.sleep(2.0)
    else:
        raise last_err
    out = np.concatenate(
        [np.asarray(r["out"]).astype(np.float32).reshape(N_LOC, C, H, W)
         for r in res.results], axis=0
    )
    return out


if __name__ == "__main__":
    rng = np.random.default_rng(0)
    x = rng.standard_normal((64, C, H, W), dtype=np.float32)
    w1 = ((rng.random((C * C * 9, 1), dtype=np.float32) - 0.5) * 0.002)
    w2 = ((rng.random((C * C * 9, 1), dtype=np.float32) - 0.5) * 0.002)
    ones = np.ones(C, np.float32)
    zeros = np.zeros(C, np.float32)
    y = kernel(x=x, bn1_g=ones, bn1_b=zeros, w1=w1, prelu1_a=np.float32(0.25),
               bn2_g=ones, bn2_b=zeros, w2=w2, prelu2_a=np.float32(0.25),
               bn3_g=ones, bn3_b=zeros, prelu3_a=np.float32(0.25))
    print("out", y.shape, y.dtype, float(np.abs(y).mean()))
